# revision 1
# baseline (speedup 1.0000x reference)
"""GCNConv-style message passing kernel for Trainium2, 8 NeuronCores.

Computes (reference semantics):
    deg  = 1 + segment_sum(edge_weight, col)           # self-loop included
    dinv = deg ** -0.5
    h    = embs @ W
    out[t] = (sum_e norm_e * h[src_e] + dinv[t]^2 * h[t]) * X[t],
             norm_e = dinv[src_e] * ew_e * dinv[t]

Device formulation (matmul commutes past the segment sum):
    embs' = dinv[:, None] * embs                        (host, fp16)
    u[t]  = sum_{e: col=t} ew_e * embs'[src_e] + embs'[t]
    out[t] = (u[t] @ W) * (dinv[t] * X[t])

Sharding: targets split across 8 cores (12500 each). Edges bucketed by
(dest-block of 128 targets, source bank of 25000 rows). Edge source rows are
fetched with dma_gather (int16 bank-local indices); per 128-edge chunk a 0/1
selection matrix S[e, t_loc] = (tloc[e] == iota) is built on DVE and
PE-matmul-accumulated into PSUM u^T[cin, t_loc]. Self loops enter via an
identity matmul of the (contiguous) target rows of embs'.
"""

import numpy as np

import concourse.bacc as bacc
import concourse.tile as tile
from concourse import mybir
from concourse.bass_utils import run_bass_kernel_spmd

P = 128


class _Cfg:
    def __init__(self, n, n_cores, bank_size, sb_group):
        self.N = n
        self.NCORES = n_cores
        self.TPC = n // n_cores              # targets per core
        assert self.TPC * n_cores == n
        self.NSB = -(-self.TPC // P)         # dest blocks of 128 per core
        self.BANK = bank_size                # gather bank rows (int16 < 32768)
        self.NBANK = -(-n // bank_size)
        assert bank_size <= 32768
        self.SB_GROUP = sb_group             # dest blocks per dma_gather


_REAL = _Cfg(n=100000, n_cores=8, bank_size=25000, sb_group=8)


def _host_prep(cfg, X, embs, W, edge_index, edge_weight):
    """Sort/bucket edges, build static chunk schedule + per-core arrays."""
    N, TPC, NSB, BANK, NBANK, NCORES = (
        cfg.N, cfg.TPC, cfg.NSB, cfg.BANK, cfg.NBANK, cfg.NCORES)

    src = np.asarray(edge_index[0], dtype=np.int64)
    col = np.asarray(edge_index[1], dtype=np.int64)
    ew = np.asarray(edge_weight, dtype=np.float64)

    deg = 1.0 + np.bincount(col, weights=ew, minlength=N)
    dinv = (1.0 / np.sqrt(deg)).astype(np.float32)

    embs16 = (dinv[:, None] * np.asarray(embs, np.float32)).astype(np.float16)
    gX = (dinv[:, None] * np.asarray(X, np.float32)).astype(np.float32)

    ew_ones = bool(np.all(np.asarray(edge_weight) == 1.0))

    core = col // TPC
    sb = (col % TPC) // P
    bank = src // BANK
    bucket = (core * NSB + sb) * NBANK + bank
    order = np.argsort(bucket, kind="stable")
    b_sorted = bucket[order]
    src_l = (src[order] - (bank[order] * BANK)).astype(np.int16)
    tl = (col[order] % TPC % P).astype(np.float32)
    ew_s = np.asarray(edge_weight, np.float32)[order]

    counts = np.bincount(bucket, minlength=NCORES * NSB * NBANK)
    counts = counts.reshape(NCORES, NSB, NBANK)
    nch = -(-counts // P)                    # ceil chunks per (core, sb, bank)
    nch = nch.max(axis=0)                    # static across cores [NSB, NBANK]
    nch[:, 0] = np.maximum(nch[:, 0], 1)     # first bucket must init PSUM

    # dest-block groups for gather granularity
    groups = [list(range(g, min(g + cfg.SB_GROUP, NSB)))
              for g in range(0, NSB, cfg.SB_GROUP)]

    # slot layout: for gi, for bank, for sb in group, chunks of (sb, bank)
    chunk_base = np.zeros((NSB, NBANK), np.int64)   # chunk index of bucket
    seg = []                                        # (gi, b) -> (chunk_off, nchunks)
    pos = 0
    for gi, sbs in enumerate(groups):
        for b in range(NBANK):
            off = pos
            for s in sbs:
                chunk_base[s, b] = pos
                pos += nch[s, b]
            seg.append((off, pos - off))
    nch_tot = pos
    slots_tot = nch_tot * P

    # scatter edges into slots
    slot_base = chunk_base * P                       # [NSB, NBANK]
    cnt_flat = counts.reshape(-1)
    starts = np.zeros_like(cnt_flat)
    np.cumsum(cnt_flat[:-1], out=starts[1:])
    rank = np.arange(len(order)) - starts[b_sorted]
    sb_s = (b_sorted // NBANK) % NSB
    bk_s = b_sorted % NBANK
    core_s = b_sorted // (NSB * NBANK)
    dest = slot_base[sb_s, bk_s] + rank

    IDX = np.zeros((NCORES, slots_tot), np.int16)
    TL = np.full((NCORES, slots_tot), -1000.0, np.float32)
    IDX[core_s, dest] = src_l
    TL[core_s, dest] = tl
    EW = None
    if not ew_ones:
        EW = np.ones((NCORES, slots_tot), np.float32)
        EW[core_s, dest] = ew_s

    # pack gather indices: per (gi,b) segment wrap-16, then replicate to 128
    idx_packed = IDX.reshape(NCORES, slots_tot // 16, 16).transpose(0, 2, 1)
    # idx i of a segment must live at [i%16, seg_col_off + i//16]; since
    # segments are slot-aligned to 128 (chunks), per-segment wrapping equals
    # global wrapping restricted to the segment's columns.
    idx_all = np.tile(idx_packed, (1, 8, 1)).astype(np.int16)  # [C,128,slots/16]

    tloc_all = TL.reshape(NCORES, nch_tot, P).transpose(0, 2, 1).copy()
    ew_all = None
    if EW is not None:
        ew_all = EW.reshape(NCORES, nch_tot, P).transpose(0, 2, 1).copy()

    iota = np.tile(np.arange(P, dtype=np.float32), (P, 1))
    ident = np.eye(P, dtype=np.float16)

    sched = dict(groups=groups, nch=nch, chunk_base=chunk_base, seg=seg,
                 nch_tot=nch_tot, ew_ones=ew_ones)
    in_maps = []
    for c in range(NCORES):
        m = dict(
            embs16=embs16,
            w32=np.asarray(W, np.float32),
            gx=np.ascontiguousarray(gX[c * TPC:(c + 1) * TPC]),
            selfrows=np.ascontiguousarray(embs16[c * TPC:(c + 1) * TPC]),
            idxall=np.ascontiguousarray(idx_all[c]),
            tlocall=np.ascontiguousarray(tloc_all[c]),
            iota=iota,
            ident=ident,
        )
        if ew_all is not None:
            m["ewall"] = np.ascontiguousarray(ew_all[c])
        in_maps.append(m)
    return sched, in_maps


def _build_program(cfg, sched):
    N, TPC, NSB, BANK, NBANK = cfg.N, cfg.TPC, cfg.NSB, cfg.BANK, cfg.NBANK
    groups, nch, chunk_base, seg, nch_tot, ew_ones = (
        sched["groups"], sched["nch"], sched["chunk_base"], sched["seg"],
        sched["nch_tot"], sched["ew_ones"])
    slots_tot = nch_tot * P

    nc = bacc.Bacc("TRN2", target_bir_lowering=False, debug=False,
                   num_devices=cfg.NCORES)
    t_embs16 = nc.dram_tensor("embs16", [N, P], mybir.dt.float16,
                              kind="ExternalInput").ap()
    t_w = nc.dram_tensor("w32", [P, P], mybir.dt.float32,
                         kind="ExternalInput").ap()
    t_gx = nc.dram_tensor("gx", [TPC, P], mybir.dt.float32,
                          kind="ExternalInput").ap()
    t_idx = nc.dram_tensor("idxall", [P, slots_tot // 16], mybir.dt.int16,
                           kind="ExternalInput").ap()
    t_tloc = nc.dram_tensor("tlocall", [P, nch_tot], mybir.dt.float32,
                            kind="ExternalInput").ap()
    t_iota = nc.dram_tensor("iota", [P, P], mybir.dt.float32,
                            kind="ExternalInput").ap()
    t_ident = nc.dram_tensor("ident", [P, P], mybir.dt.float16,
                             kind="ExternalInput").ap()
    t_selfrows = nc.dram_tensor("selfrows", [TPC, P], mybir.dt.float16,
                                kind="ExternalInput").ap()
    t_ew = None
    if not ew_ones:
        t_ew = nc.dram_tensor("ewall", [P, nch_tot], mybir.dt.float32,
                              kind="ExternalInput").ap()
    t_out = nc.dram_tensor("out", [TPC, P], mybir.dt.float32,
                           kind="ExternalOutput").ap()

    with tile.TileContext(nc) as tc:
        with tc.tile_pool(name="const", bufs=1) as cpool, \
             tc.tile_pool(name="meta", bufs=1) as mpool, \
             tc.tile_pool(name="gpool", bufs=6) as gpool, \
             tc.tile_pool(name="spool", bufs=6) as spool, \
             tc.tile_pool(name="xfer", bufs=4) as xfer, \
             tc.tile_pool(name="psu", bufs=4, space="PSUM") as psu, \
             tc.tile_pool(name="psb", bufs=4, space="PSUM") as psb:

            iota_t = cpool.tile([P, P], mybir.dt.float32)
            nc.sync.dma_start(out=iota_t, in_=t_iota)
            ident_t = cpool.tile([P, P], mybir.dt.float16)
            nc.sync.dma_start(out=ident_t, in_=t_ident)
            w_t = cpool.tile([P, P], mybir.dt.float32)
            nc.sync.dma_start(out=w_t, in_=t_w)
            idx_t = mpool.tile([P, slots_tot // 16], mybir.dt.int16)
            nc.sync.dma_start(out=idx_t, in_=t_idx)
            tloc_t = mpool.tile([P, nch_tot], mybir.dt.float32)
            nc.sync.dma_start(out=tloc_t, in_=t_tloc)
            ew_t = None
            if t_ew is not None:
                ew_t = mpool.tile([P, nch_tot], mybir.dt.float32)
                nc.sync.dma_start(out=ew_t, in_=t_ew)

            for gi, sbs in enumerate(groups):
                g_tiles = []
                for b in range(NBANK):
                    off, nseg = seg[gi * NBANK + b]
                    if nseg == 0:
                        g_tiles.append(None)
                        continue
                    g_t = gpool.tile([P, nseg, P], mybir.dt.float16, tag="g")
                    rows = min(BANK, N - b * BANK)
                    nc.gpsimd.dma_gather(
                        out_ap=g_t[:, :, :],
                        in_ap=t_embs16[b * BANK: b * BANK + rows, :],
                        idxs_ap=idx_t[:, off * 8:(off + nseg) * 8],
                        num_idxs=nseg * P,
                        num_idxs_reg=nseg * P,
                        elem_size=P,
                        single_packet=False,
                    )
                    g_tiles.append(g_t)

                for s in sbs:
                    t0 = s * P
                    tw = min(P, TPC - t0)
                    psum_u = psu.tile([P, P], mybir.dt.float32, space="PSUM")
                    first = True
                    for b in range(NBANK):
                        off, nseg = seg[gi * NBANK + b]
                        for j in range(int(nch[s, b])):
                            ch = int(chunk_base[s, b]) + j
                            s_t = spool.tile([P, P], mybir.dt.float16, tag="s")
                            nc.vector.tensor_tensor(
                                out=s_t, in0=iota_t,
                                in1=tloc_t[:, ch:ch + 1].to_broadcast([P, P]),
                                op=mybir.AluOpType.is_equal,
                            )
                            if ew_t is not None:
                                s2 = spool.tile([P, P], mybir.dt.float16,
                                                tag="s2")
                                nc.vector.tensor_tensor(
                                    out=s2, in0=s_t,
                                    in1=ew_t[:, ch:ch + 1].to_broadcast([P, P]),
                                    op=mybir.AluOpType.mult,
                                )
                                s_t = s2
                            nc.tensor.matmul(
                                out=psum_u[:, :],
                                lhsT=g_tiles[b][:, ch - off, :],
                                rhs=s_t,
                                start=first, stop=False,
                            )
                            first = False
                    assert not first
                    # self loops: += embs'[t]^T via identity matmul
                    self_t = xfer.tile([P, P], mybir.dt.float16, tag="self")
                    nc.sync.dma_start(
                        out=self_t[:tw, :],
                        in_=t_selfrows[t0:t0 + tw, :],
                    )
                    nc.tensor.matmul(
                        out=psum_u[:, :tw],
                        lhsT=self_t[:tw, :],
                        rhs=ident_t[:tw, :tw],
                        start=False, stop=True,
                    )

                    u_t = xfer.tile([P, P], mybir.dt.float32, tag="u")
                    nc.vector.tensor_copy(out=u_t[:, :tw], in_=psum_u[:, :tw])

                    psum_o = psb.tile([P, P], mybir.dt.float32, space="PSUM")
                    nc.tensor.matmul(out=psum_o[:tw, :], lhsT=u_t[:, :tw],
                                     rhs=w_t, start=True, stop=True)

                    gx_t = xfer.tile([P, P], mybir.dt.float32, tag="gx")
                    nc.sync.dma_start(out=gx_t[:tw, :],
                                      in_=t_gx[t0:t0 + tw, :])
                    o_t = xfer.tile([P, P], mybir.dt.float32, tag="o")
                    nc.vector.tensor_tensor(out=o_t[:tw, :],
                                            in0=psum_o[:tw, :],
                                            in1=gx_t[:tw, :],
                                            op=mybir.AluOpType.mult)
                    nc.sync.dma_start(out=t_out[t0:t0 + tw, :],
                                      in_=o_t[:tw, :])
    nc.compile()
    return nc


def kernel(X, embs, W, edge_index, edge_weight):
    cfg = _REAL
    sched, in_maps = _host_prep(cfg, X, embs, W, edge_index, edge_weight)
    nc = _build_program(cfg, sched)
    res = run_bass_kernel_spmd(nc, in_maps, list(range(cfg.NCORES)))
    out = np.concatenate([res.results[c]["out"] for c in range(cfg.NCORES)],
                         axis=0)
    return out.astype(np.float32)



# revision 9
# speedup vs baseline: 1.7188x; 1.7188x over previous
"""GCNConv-style message passing kernel for Trainium2, 8 NeuronCores.

Reference semantics:
    deg  = 1 + segment_sum(edge_weight, col)           # self-loop included
    dinv = deg ** -0.5
    h    = embs @ W
    out[t] = (sum_e norm_e * h[src_e] + dinv[t]^2 * h[t]) * X[t],
             norm_e = dinv[src_e] * ew_e * dinv[t]

Device formulation (matmul commutes past the segment sum):
    embs8 = e3m4(SCALE * dinv[:, None] * embs)          (host, fp8 e3m4)
    u[t]  = sum_{e: col=t} ew_e * embs8[src_e]          (self loop folded in
                                                         as an extra edge)
    out[t] = (fp16(u[t]) @ W16) * (dinv[t] * X[t] / SCALE)

Sharding: targets split across 8 cores (12500 each). Edges (incl. self
edges) are grouped by (dest-block-group of 16x128 targets, source bank of
32768 rows) into slot segments; within a segment, per-(dest-block, bank)
slot spans are sized max-over-cores so the chunk schedule is shared SPMD.
Edge source rows are fetched with a raw 128-byte fp8 dma_gather (256B DRAM
stride). Per 128-slot chunk a 0/1 selection matrix S[e, t_loc] is built on
DVE via tensor_scalar is_equal (4x perf mode) and PE-matmul-accumulated
into PSUM u^T[cin, t_loc]; chunks straddling dest-block boundaries emit one
masked S per covered block. ACT copies PSUM->SBUF (fp16), PE applies W,
DVE multiplies by the gating and the result is written back once as fp16.
"""

import sys

import numpy as np
import ml_dtypes

import concourse.bacc as bacc
import concourse.tile as tile
from concourse import mybir
from concourse.bass import exact_div
from concourse.bass_utils import run_bass_kernel_spmd

P = 128
E3M4 = ml_dtypes.float8_e3m4
SCALE = 4.0
E3M4_MAX = 15.5


class _Cfg:
    def __init__(self, n, n_cores, sb_group):
        self.N = n
        self.NCORES = n_cores
        self.TPC = n // n_cores              # targets per core
        assert self.TPC * n_cores == n
        self.NSB = -(-self.TPC // P)         # dest blocks of 128 per core
        self.BANK = 32768                    # gather bank rows (int16 idx)
        self.NV = n + self.TPC               # rows incl. per-core self region
        self.NBANK = -(-self.NV // self.BANK)
        self.SBG = sb_group                  # dest blocks per group
        self.NG = -(-self.NSB // sb_group)


_REAL = _Cfg(n=100000, n_cores=8, sb_group=16)


def _dma_gather_raw(gp, out_ap, in_ap, idxs_ap, num_idxs, elem_size,
                    elem_step, single_packet=False):
    """bass dma_gather clone (DRAM src, non-transpose) without the
    elem_size%256B restriction; elem_step sets the 256B-unit DRAM stride."""
    assert idxs_ap.dtype == mybir.dt.int16
    assert in_ap.dtype == out_ap.dtype
    assert in_ap.ap[-1][1] == elem_size
    assert out_ap.ap[-1][1] == elem_size
    assert out_ap.ap[0][1] * out_ap.ap[1][1] == ((num_idxs + 127) // 128) * 128
    assert in_ap.ap[0][0] == elem_step
    stride_bytes_256 = exact_div(elem_step * mybir.dt.size(in_ap.dtype), 256)
    assert 0 < stride_bytes_256 < 256
    _in_ap = gp.lower_ap_dma(in_ap, for_custom_bir_dma=True)
    _idxs_ap = gp.lower_ap(idxs_ap)
    _out_ap = gp.lower_ap(out_ap)
    return gp.add_instruction(
        mybir.InstDMAGatherAnt(
            name=gp.bass.get_next_instruction_name(),
            ins=[*_in_ap, _idxs_ap, gp.lower_val_access(gp.to_reg(num_idxs))],
            outs=[_out_ap],
            transpose=False,
            num_idxs=num_idxs,
            elem_size=elem_size,
            stride_bytes_256=stride_bytes_256,
            gen_mode=0,
            single_packet=single_packet,
            queue_num=0,
            sbuf_tokens_per_rank=0,
            sbuf_free_dim_per_rank=0,
            sbuf_free_dim_pad_per_rank=0,
            sbuf_byte_offset=0,
        )
    )


def _host_prep(cfg, X, embs, W, edge_index, edge_weight):
    """Build fp8 embs table, slot layout, chunk schedule, per-core arrays."""
    N, TPC, NSB, BANK, NBANK = cfg.N, cfg.TPC, cfg.NSB, cfg.BANK, cfg.NBANK
    NC, SBG, NG = cfg.NCORES, cfg.SBG, cfg.NG

    src = np.asarray(edge_index[0], dtype=np.int64)
    col = np.asarray(edge_index[1], dtype=np.int64)
    ew = np.asarray(edge_weight, dtype=np.float32)
    ew_ones = bool(np.all(ew == 1.0))
    E = src.shape[0]

    # self loops as ordinary edges; their source points into a per-core
    # virtual row region [N, N+TPC) so the (block, bank) slot layout is
    # identical across cores.
    loop = np.arange(N, dtype=np.int64)
    src_a = np.concatenate([src, N + (loop % TPC)])
    col_a = np.concatenate([col, loop])
    ew_a = np.concatenate([ew, np.ones(N, np.float32)])

    deg = 1.0 + np.bincount(col, weights=ew.astype(np.float64), minlength=N)
    dinv = (1.0 / np.sqrt(deg)).astype(np.float32)

    embs_s = dinv[:, None] * np.asarray(embs, np.float32)
    embs8 = np.zeros((cfg.NV, 256), E3M4)
    embs8[:N, :P] = np.clip(embs_s * SCALE, -E3M4_MAX, E3M4_MAX).astype(E3M4)
    gx = (dinv[:, None] * np.asarray(X, np.float32)) * (1.0 / SCALE)

    # bucket keys
    core = col_a // TPC
    sb = (col_a % TPC) // P                  # 0..NSB-1
    grp = sb // SBG
    sbl = sb - grp * SBG                     # block local to group
    bank = src_a // BANK

    key = ((core * NG + grp) * NBANK + bank) * SBG + sbl
    nkey = NC * NG * NBANK * SBG
    counts = np.bincount(key, minlength=nkey).reshape(NC, NG, NBANK, SBG)
    M = counts.max(axis=0)                   # shared span sizes [NG,NBANK,SBG]

    # shared slot layout: segments (g,b) in order, blocks in order inside,
    # each segment padded to a 128 multiple.
    blk_off = np.zeros((NG, NBANK, SBG), np.int64)   # block span start
    seg_base = np.zeros((NG, NBANK), np.int64)       # segment slot base
    nch = np.zeros((NG, NBANK), np.int64)            # chunks per segment
    pos = 0
    for g in range(NG):
        nb = min(SBG, NSB - g * SBG)
        for b in range(NBANK):
            seg_base[g, b] = pos
            off = 0
            for s in range(nb):
                blk_off[g, b, s] = off
                off += M[g, b, s]
            nch[g, b] = -(-off // P)
            pos += nch[g, b] * P
    slots_tot = pos

    # chunk/block overlap -> tloc columns, in device processing order
    colid = {}
    sched = []                               # [g][s_local] -> [(b, ch, col)]
    ncols = 0
    for g in range(NG):
        nb = min(SBG, NSB - g * SBG)
        gsched = []
        for s in range(nb):
            lst = []
            for b in range(NBANK):
                lo = blk_off[g, b, s]
                hi = lo + M[g, b, s]
                if hi == lo:
                    continue
                c0, c1 = lo // P, (hi - 1) // P
                for c in range(c0, c1 + 1):
                    colid[(g, b, c, s)] = ncols
                    lst.append((b, int(c), ncols))
                    ncols += 1
            assert lst, f"block {g},{s} has no slots"
            gsched.append(lst)
        sched.append(gsched)

    # per-edge destination slots
    cnt_flat = counts.reshape(-1)
    order = np.argsort(key, kind="stable")
    starts = np.zeros_like(cnt_flat)
    np.cumsum(cnt_flat[:-1], out=starts[1:])
    k_sorted = key[order]
    rank = np.arange(len(order)) - starts[k_sorted]
    g_s, b_s = grp[order], bank[order]
    sbl_s, core_s = sbl[order], core[order]
    dest = (seg_base[g_s, b_s] + blk_off[g_s, b_s, sbl_s] + rank)
    chunk_s = (dest - seg_base[g_s, b_s]) // P    # segment-local chunk
    lane_s = dest % P
    tl_s = (col_a[order] % TPC % P).astype(np.float32)
    src_l = (src_a[order] - b_s * BANK).astype(np.int16)
    ew_s = ew_a[order]

    # column index per edge (vectorized via dict -> array)
    ckey = ((g_s * NBANK + b_s) * (slots_tot // P + 1) + chunk_s) * SBG + sbl_s
    uk, inv = np.unique(ckey, return_inverse=True)
    uk_col = np.empty(len(uk), np.int64)
    for i, k in enumerate(uk):
        sblk = k % SBG
        k //= SBG
        ch = k % (slots_tot // P + 1)
        k //= (slots_tot // P + 1)
        b = k % NBANK
        g = k // NBANK
        uk_col[i] = colid[(g, b, ch, sblk)]
    col_e = uk_col[inv]

    IDX = np.zeros((NC, slots_tot), np.int16)
    TLOC = np.full((NC, P, ncols), -100.0, np.float32)
    IDX[core_s, dest] = src_l
    TLOC[core_s, lane_s, col_e] = tl_s
    EWC = None
    if not ew_ones:
        EWC = np.ones((NC, P, ncols), np.float32)
        EWC[core_s, lane_s, col_e] = ew_s

    # pack gather indices wrap-16, replicate to 128 partitions
    idx_packed = IDX.reshape(NC, slots_tot // 16, 16).transpose(0, 2, 1)
    idx_all = np.tile(idx_packed, (1, 8, 1)).astype(np.int16)

    # partition-major gx / out layout: pm[p, s*128 + c] = row (s*128+p)
    npad = NSB * P
    gx_pm = np.zeros((NC, P, npad), np.float32)
    for c in range(NC):
        gxc = gx[c * TPC:(c + 1) * TPC]
        gxc = np.concatenate(
            [gxc, np.zeros((npad - TPC, P), np.float32)], axis=0)
        gx_pm[c] = gxc.reshape(NSB, P, P).transpose(1, 0, 2).reshape(P, npad)

    iota16 = np.tile(np.arange(P, dtype=np.float16), (P, 1))
    w16 = np.asarray(W, np.float16)

    meta = dict(sched=sched, nch=nch, seg_base=seg_base, slots_tot=slots_tot,
                ncols=ncols, ew_ones=ew_ones)
    in_maps = []
    for c in range(NC):
        e8 = embs8.copy()
        e8[N:N + TPC] = embs8[c * TPC:(c + 1) * TPC]
        m = dict(
            embs8=e8,
            w16=w16,
            gx=np.ascontiguousarray(gx_pm[c]),
            idxall=np.ascontiguousarray(idx_all[c]),
            tlocall=np.ascontiguousarray(TLOC[c]),
            iota=iota16,
        )
        if EWC is not None:
            m["ewall"] = np.ascontiguousarray(EWC[c])
        in_maps.append(m)
    return meta, in_maps


def _build_program(cfg, meta):
    N, TPC, NSB, BANK, NBANK = cfg.N, cfg.TPC, cfg.NSB, cfg.BANK, cfg.NBANK
    SBG, NG = cfg.SBG, cfg.NG
    sched, nch, seg_base = meta["sched"], meta["nch"], meta["seg_base"]
    slots_tot, ncols, ew_ones = (meta["slots_tot"], meta["ncols"],
                                 meta["ew_ones"])
    npad = NSB * P

    nc = bacc.Bacc("TRN2", target_bir_lowering=False, debug=False,
                   num_devices=cfg.NCORES)
    t_embs8 = nc.dram_tensor("embs8", [cfg.NV, 256], mybir.dt.float8e3,
                             kind="ExternalInput").ap()
    t_w = nc.dram_tensor("w16", [P, P], mybir.dt.float16,
                         kind="ExternalInput").ap()
    t_gx = nc.dram_tensor("gx", [P, npad], mybir.dt.float32,
                          kind="ExternalInput").ap()
    t_idx = nc.dram_tensor("idxall", [P, slots_tot // 16], mybir.dt.int16,
                           kind="ExternalInput").ap()
    t_tloc = nc.dram_tensor("tlocall", [P, ncols], mybir.dt.float32,
                            kind="ExternalInput").ap()
    t_iota = nc.dram_tensor("iota", [P, P], mybir.dt.float16,
                            kind="ExternalInput").ap()
    t_ew = None
    if not ew_ones:
        t_ew = nc.dram_tensor("ewall", [P, ncols], mybir.dt.float32,
                              kind="ExternalInput").ap()
    t_out = nc.dram_tensor("out", [P, npad], mybir.dt.float16,
                           kind="ExternalOutput").ap()

    with tile.TileContext(nc) as tc:
        with tc.tile_pool(name="const", bufs=1) as cpool, \
             tc.tile_pool(name="gpool", bufs=2) as gpool, \
             tc.tile_pool(name="spool", bufs=8) as spool, \
             tc.tile_pool(name="xfer", bufs=4) as xfer, \
             tc.tile_pool(name="psu", bufs=4, space="PSUM") as psu, \
             tc.tile_pool(name="psb", bufs=4, space="PSUM") as psb:

            iota_t = cpool.tile([P, P], mybir.dt.float16)
            nc.sync.dma_start(out=iota_t, in_=t_iota)
            w_t = cpool.tile([P, P], mybir.dt.float16)
            nc.sync.dma_start(out=w_t, in_=t_w)
            gx_t = cpool.tile([P, npad], mybir.dt.float32)
            nc.sync.dma_start(out=gx_t, in_=t_gx)
            idx_t = cpool.tile([P, slots_tot // 16], mybir.dt.int16)
            nc.sync.dma_start(out=idx_t, in_=t_idx)
            tloc_t = cpool.tile([P, ncols], mybir.dt.float32)
            nc.sync.dma_start(out=tloc_t, in_=t_tloc)
            ew_t = None
            if t_ew is not None:
                ew_t = cpool.tile([P, ncols], mybir.dt.float32)
                nc.sync.dma_start(out=ew_t, in_=t_ew)
            out_t = cpool.tile([P, npad], mybir.dt.float16)

            for g in range(NG):
                nb = min(SBG, NSB - g * SBG)
                g_tiles = []
                for b in range(NBANK):
                    nseg = int(nch[g, b])
                    if nseg == 0:
                        g_tiles.append(None)
                        continue
                    g_t = gpool.tile([P, nseg, P], mybir.dt.float8e3,
                                     tag=f"g{b}")
                    rows = min(BANK, cfg.NV - b * BANK)
                    off = int(seg_base[g, b]) // 16
                    _dma_gather_raw(
                        nc.gpsimd, g_t[:, :, :],
                        t_embs8[b * BANK: b * BANK + rows, 0:P],
                        idx_t[:, off:off + nseg * 8],
                        num_idxs=nseg * P, elem_size=P, elem_step=256,
                    )
                    g_tiles.append(g_t)

                for s in range(nb):
                    cols = sched[g][s]
                    psum_u = psu.tile([P, P], mybir.dt.float32, space="PSUM")
                    nlast = len(cols) - 1
                    for j, (b, ch, cid) in enumerate(cols):
                        s_t = spool.tile([P, P], mybir.dt.float16, tag="s")
                        nc.vector.tensor_scalar(
                            out=s_t, in0=iota_t,
                            scalar1=tloc_t[:, cid:cid + 1], scalar2=None,
                            op0=mybir.AluOpType.is_equal,
                        )
                        if ew_t is not None:
                            s2 = spool.tile([P, P], mybir.dt.float16,
                                            tag="s2")
                            nc.vector.tensor_scalar(
                                out=s2, in0=s_t,
                                scalar1=ew_t[:, cid:cid + 1], scalar2=None,
                                op0=mybir.AluOpType.mult,
                            )
                            s_t = s2
                        nc.tensor.matmul(
                            out=psum_u[:, :],
                            lhsT=g_tiles[b][:, ch, :],
                            rhs=s_t,
                            start=(j == 0), stop=(j == nlast),
                        )

                    u16 = xfer.tile([P, P], mybir.dt.float16, tag="u")
                    nc.scalar.activation(
                        out=u16, in_=psum_u,
                        func=mybir.ActivationFunctionType.Copy)

                    psum_o = psb.tile([P, P], mybir.dt.float32, space="PSUM")
                    nc.tensor.matmul(out=psum_o, lhsT=u16, rhs=w_t,
                                     start=True, stop=True)

                    sg = (g * SBG + s) * P
                    nc.vector.tensor_tensor(
                        out=out_t[:, sg:sg + P], in0=psum_o,
                        in1=gx_t[:, sg:sg + P], op=mybir.AluOpType.mult)

            nc.sync.dma_start(out=t_out, in_=out_t)
    nc.compile()
    return nc


def _unshard(cfg, res):
    TPC, NSB = cfg.TPC, cfg.NSB
    outs = []
    for c in range(cfg.NCORES):
        o = res.results[c]["out"]                       # [P, NSB*P] fp16
        o = np.asarray(o, np.float32).reshape(P, NSB, P)
        o = o.transpose(1, 0, 2).reshape(NSB * P, P)[:TPC]
        outs.append(o)
    return np.concatenate(outs, axis=0).astype(np.float32)


def kernel(X, embs, W, edge_index, edge_weight):
    cfg = _REAL
    meta, in_maps = _host_prep(cfg, X, embs, W, edge_index, edge_weight)
    nc = _build_program(cfg, meta)
    res = run_bass_kernel_spmd(nc, in_maps, list(range(cfg.NCORES)))
    return _unshard(cfg, res)


# revision 11
# speedup vs baseline: 2.5982x; 1.5116x over previous
"""GCNConv-style message passing kernel for Trainium2, 8 NeuronCores.

Reference semantics:
    deg  = 1 + segment_sum(edge_weight, col)           # self-loop included
    dinv = deg ** -0.5
    h    = embs @ W
    out[t] = (sum_e norm_e * h[src_e] + dinv[t]^2 * h[t]) * X[t],
             norm_e = dinv[src_e] * ew_e * dinv[t]

Device formulation (matmul commutes past the segment sum):
    embs8 = e3m4(SCALE * dinv[:, None] * embs)          (host, fp8 e3m4)
    u[t]  = sum_{e: col=t} ew_e * embs8[src_e]          (self loop folded in
                                                         as an extra edge)
    out[t] = (fp16(u[t]) @ W16) * fp16(dinv[t] * X[t] / SCALE)

Sharding: targets split across 8 cores (12500 each). Edges (incl. self
edges, whose sources point into a per-core virtual row region so the
layout is core-uniform) are grouped by (dest-block-group of SBG x 128
targets, source bank of 32768 rows) into slot segments; within a segment,
per-(dest-block, bank) slot spans are sized max-over-cores so the chunk
schedule is shared SPMD. Source rows are fetched with a raw 128-byte fp8
dma_gather (256B DRAM stride). Selection matrices S[e, t_loc] for BATCH
chunks at a time are built by ONE DVE tensor_tensor is_equal (2x fp16
mode) into a column-major [128, 128, BATCH] tile; chunk j's S is the
stride-BATCH slice [:, :, j], consumed by PE matmuls accumulating PSUM
u^T[cin, t_loc]. Chunks straddling dest-block boundaries emit one masked
S column per covered block. ACT copies PSUM->SBUF as fp16, PE applies W
into 4-block-wide PSUM tiles, DVE multiplies by the gating (deferred one
unit to avoid stalls) and the result is written back once as fp16.
"""

import numpy as np
import ml_dtypes

import concourse.bacc as bacc
import concourse.tile as tile
from concourse import mybir
from concourse.bass import exact_div
from concourse.bass_utils import run_bass_kernel_spmd

P = 128
BATCH = 16            # S columns built per DVE instruction
UNIT = 4              # dest blocks per output psum tile
E3M4 = ml_dtypes.float8_e3m4
SCALE = 4.0
E3M4_MAX = 15.5


class _Cfg:
    def __init__(self, n, n_cores, sb_group):
        self.N = n
        self.NCORES = n_cores
        self.TPC = n // n_cores              # targets per core
        assert self.TPC * n_cores == n
        self.NSB = -(-self.TPC // P)         # dest blocks of 128 per core
        self.BANK = 32768                    # gather bank rows (int16 idx)
        self.NV = n + self.TPC               # rows incl. per-core self region
        self.NBANK = -(-self.NV // self.BANK)
        self.SBG = sb_group                  # dest blocks per group
        self.NG = -(-self.NSB // sb_group)


_REAL = _Cfg(n=100000, n_cores=8, sb_group=8)


def _dma_gather_raw(gp, out_ap, in_ap, idxs_ap, num_idxs, elem_size,
                    elem_step, single_packet=False):
    """bass dma_gather clone (DRAM src, non-transpose) without the
    elem_size%256B restriction; elem_step sets the 256B-unit DRAM stride."""
    assert idxs_ap.dtype == mybir.dt.int16
    assert in_ap.dtype == out_ap.dtype
    assert in_ap.ap[-1][1] == elem_size
    assert out_ap.ap[-1][1] == elem_size
    assert out_ap.ap[0][1] * out_ap.ap[1][1] == ((num_idxs + 127) // 128) * 128
    assert in_ap.ap[0][0] == elem_step
    stride_bytes_256 = exact_div(elem_step * mybir.dt.size(in_ap.dtype), 256)
    assert 0 < stride_bytes_256 < 256
    _in_ap = gp.lower_ap_dma(in_ap, for_custom_bir_dma=True)
    _idxs_ap = gp.lower_ap(idxs_ap)
    _out_ap = gp.lower_ap(out_ap)
    return gp.add_instruction(
        mybir.InstDMAGatherAnt(
            name=gp.bass.get_next_instruction_name(),
            ins=[*_in_ap, _idxs_ap, gp.lower_val_access(gp.to_reg(num_idxs))],
            outs=[_out_ap],
            transpose=False,
            num_idxs=num_idxs,
            elem_size=elem_size,
            stride_bytes_256=stride_bytes_256,
            gen_mode=0,
            single_packet=single_packet,
            queue_num=0,
            sbuf_tokens_per_rank=0,
            sbuf_free_dim_per_rank=0,
            sbuf_free_dim_pad_per_rank=0,
            sbuf_byte_offset=0,
        )
    )


def _bcast_mid(ap, rep):
    """[P, k] AP -> [P, rep, k] with a stride-0 middle dim."""
    cls = type(ap)
    new = [list(ap.ap[0]), [0, rep], list(ap.ap[1])]
    return cls(ap.tensor, ap.offset, new)


def _host_prep(cfg, X, embs, W, edge_index, edge_weight):
    """Build fp8 embs table, slot layout, chunk schedule, per-core arrays."""
    N, TPC, NSB, BANK, NBANK = cfg.N, cfg.TPC, cfg.NSB, cfg.BANK, cfg.NBANK
    NC, SBG, NG = cfg.NCORES, cfg.SBG, cfg.NG

    src = np.asarray(edge_index[0], dtype=np.int64)
    col = np.asarray(edge_index[1], dtype=np.int64)
    ew = np.asarray(edge_weight, dtype=np.float32)
    ew_ones = bool(np.all(ew == 1.0))

    # self loops as ordinary edges; their source points into a per-core
    # virtual row region [N, N+TPC) so the (block, bank) slot layout is
    # identical across cores.
    loop = np.arange(N, dtype=np.int64)
    src_a = np.concatenate([src, N + (loop % TPC)])
    col_a = np.concatenate([col, loop])
    ew_a = np.concatenate([ew, np.ones(N, np.float32)])

    deg = 1.0 + np.bincount(col, weights=ew.astype(np.float64), minlength=N)
    dinv = (1.0 / np.sqrt(deg)).astype(np.float32)

    embs_s = dinv[:, None] * np.asarray(embs, np.float32)
    embs8 = np.zeros((cfg.NV, 256), E3M4)
    embs8[:N, :P] = np.clip(embs_s * SCALE, -E3M4_MAX, E3M4_MAX).astype(E3M4)
    gx = (dinv[:, None] * np.asarray(X, np.float32)) * (1.0 / SCALE)

    # bucket keys
    core = col_a // TPC
    sb = (col_a % TPC) // P                  # 0..NSB-1
    grp = sb // SBG
    sbl = sb - grp * SBG                     # block local to group
    bank = src_a // BANK

    key = ((core * NG + grp) * NBANK + bank) * SBG + sbl
    nkey = NC * NG * NBANK * SBG
    counts = np.bincount(key, minlength=nkey).reshape(NC, NG, NBANK, SBG)
    M = counts.max(axis=0)                   # shared span sizes [NG,NBANK,SBG]

    # shared slot layout: segments (g,b) in order, blocks in order inside,
    # each segment padded to a 128 multiple.
    blk_off = np.zeros((NG, NBANK, SBG), np.int64)   # block span start
    seg_base = np.zeros((NG, NBANK), np.int64)       # segment slot base
    nch = np.zeros((NG, NBANK), np.int64)            # chunks per segment
    pos = 0
    for g in range(NG):
        nb = min(SBG, NSB - g * SBG)
        for b in range(NBANK):
            seg_base[g, b] = pos
            off = 0
            for s in range(nb):
                blk_off[g, b, s] = off
                off += M[g, b, s]
            nch[g, b] = -(-off // P)
            pos += nch[g, b] * P
    slots_tot = pos

    # chunk/block overlap -> tloc columns, in device processing order
    colid = {}
    sched = []                               # [g][s_local] -> [(b, ch, col)]
    ncols = 0
    for g in range(NG):
        nb = min(SBG, NSB - g * SBG)
        gsched = []
        for s in range(nb):
            lst = []
            for b in range(NBANK):
                lo = blk_off[g, b, s]
                hi = lo + M[g, b, s]
                if hi == lo:
                    continue
                c0, c1 = lo // P, (hi - 1) // P
                for c in range(c0, c1 + 1):
                    colid[(g, b, c, s)] = ncols
                    lst.append((b, int(c), ncols))
                    ncols += 1
            assert lst, f"block {g},{s} has no slots"
            gsched.append(lst)
        sched.append(gsched)
    ncols_pad = -(-ncols // BATCH) * BATCH

    # per-edge destination slots
    cnt_flat = counts.reshape(-1)
    order = np.argsort(key, kind="stable")
    starts = np.zeros_like(cnt_flat)
    np.cumsum(cnt_flat[:-1], out=starts[1:])
    k_sorted = key[order]
    rank = np.arange(len(order)) - starts[k_sorted]
    g_s, b_s = grp[order], bank[order]
    sbl_s, core_s = sbl[order], core[order]
    dest = (seg_base[g_s, b_s] + blk_off[g_s, b_s, sbl_s] + rank)
    chunk_s = (dest - seg_base[g_s, b_s]) // P    # segment-local chunk
    lane_s = dest % P
    tl_s = (col_a[order] % TPC % P).astype(np.float32)
    src_l = (src_a[order] - b_s * BANK).astype(np.int16)
    ew_s = ew_a[order]

    # column index per edge (vectorized via dict -> array)
    mchunk = slots_tot // P + 1
    ckey = ((g_s * NBANK + b_s) * mchunk + chunk_s) * SBG + sbl_s
    uk, inv = np.unique(ckey, return_inverse=True)
    uk_col = np.empty(len(uk), np.int64)
    for i, k in enumerate(uk):
        sblk = k % SBG
        k //= SBG
        ch = k % mchunk
        k //= mchunk
        b = k % NBANK
        g = k // NBANK
        uk_col[i] = colid[(g, b, ch, sblk)]
    col_e = uk_col[inv]

    IDX = np.zeros((NC, slots_tot), np.int16)
    TLOC = np.full((NC, P, ncols_pad), -100.0, np.float16)
    IDX[core_s, dest] = src_l
    TLOC[core_s, lane_s, col_e] = tl_s
    EWC = None
    if not ew_ones:
        EWC = np.ones((NC, P, ncols_pad), np.float16)
        EWC[core_s, lane_s, col_e] = ew_s

    # pack gather indices wrap-16, replicate to 128 partitions
    idx_packed = IDX.reshape(NC, slots_tot // 16, 16).transpose(0, 2, 1)
    idx_all = np.tile(idx_packed, (1, 8, 1)).astype(np.int16)

    # partition-major gx layout: pm[p, s*128 + c] = row (s*128+p)
    npad = NSB * P
    gx_pm = np.zeros((NC, P, npad), np.float16)
    for c in range(NC):
        gxc = gx[c * TPC:(c + 1) * TPC]
        gxc = np.concatenate(
            [gxc, np.zeros((npad - TPC, P), np.float32)], axis=0)
        gx_pm[c] = gxc.reshape(NSB, P, P).transpose(1, 0, 2).reshape(
            P, npad).astype(np.float16)

    # iotacm[p, t*BATCH + j] = t  (column-major S layout constant)
    iotacm = np.repeat(np.arange(P, dtype=np.float16), BATCH)[None, :]
    iotacm = np.tile(iotacm, (P, 1))
    w16 = np.asarray(W, np.float16)

    meta = dict(sched=sched, nch=nch, seg_base=seg_base, slots_tot=slots_tot,
                ncols=ncols, ncols_pad=ncols_pad, ew_ones=ew_ones)
    in_maps = []
    for c in range(NC):
        e8 = embs8.copy()
        e8[N:N + TPC] = embs8[c * TPC:(c + 1) * TPC]
        m = dict(
            embs8=e8,
            w16=w16,
            gx=np.ascontiguousarray(gx_pm[c]),
            idxall=np.ascontiguousarray(idx_all[c]),
            tlocall=np.ascontiguousarray(TLOC[c].reshape(P, ncols_pad)),
            iotacm=iotacm,
        )
        if EWC is not None:
            m["ewall"] = np.ascontiguousarray(EWC[c].reshape(P, ncols_pad))
        in_maps.append(m)
    return meta, in_maps


def _build_program(cfg, meta):
    N, TPC, NSB, BANK, NBANK = cfg.N, cfg.TPC, cfg.NSB, cfg.BANK, cfg.NBANK
    SBG, NG = cfg.SBG, cfg.NG
    sched, nch, seg_base = meta["sched"], meta["nch"], meta["seg_base"]
    slots_tot, ncols_pad, ew_ones = (meta["slots_tot"], meta["ncols_pad"],
                                     meta["ew_ones"])
    npad = NSB * P

    nc = bacc.Bacc("TRN2", target_bir_lowering=False, debug=False,
                   num_devices=cfg.NCORES)
    t_embs8 = nc.dram_tensor("embs8", [cfg.NV, 256], mybir.dt.float8e3,
                             kind="ExternalInput").ap()
    t_w = nc.dram_tensor("w16", [P, P], mybir.dt.float16,
                         kind="ExternalInput").ap()
    t_gx = nc.dram_tensor("gx", [P, npad], mybir.dt.float16,
                          kind="ExternalInput").ap()
    t_idx = nc.dram_tensor("idxall", [P, slots_tot // 16], mybir.dt.int16,
                           kind="ExternalInput").ap()
    t_tloc = nc.dram_tensor("tlocall", [P, ncols_pad], mybir.dt.float16,
                            kind="ExternalInput").ap()
    t_iotacm = nc.dram_tensor("iotacm", [P, P * BATCH], mybir.dt.float16,
                              kind="ExternalInput").ap()
    t_ew = None
    if not ew_ones:
        t_ew = nc.dram_tensor("ewall", [P, ncols_pad], mybir.dt.float16,
                              kind="ExternalInput").ap()
    t_out = nc.dram_tensor("out", [P, npad], mybir.dt.float16,
                           kind="ExternalOutput").ap()

    with tile.TileContext(nc) as tc:
        with tc.tile_pool(name="const", bufs=1) as cpool, \
             tc.tile_pool(name="gpool", bufs=2) as gpool, \
             tc.tile_pool(name="spool", bufs=3) as spool, \
             tc.tile_pool(name="xfer", bufs=4) as xfer, \
             tc.tile_pool(name="psu", bufs=4, space="PSUM") as psu, \
             tc.tile_pool(name="psb", bufs=3, space="PSUM") as psb:

            iotacm_t = cpool.tile([P, P, BATCH], mybir.dt.float16)
            nc.sync.dma_start(out=iotacm_t, in_=t_iotacm)
            w_t = cpool.tile([P, P], mybir.dt.float16)
            nc.sync.dma_start(out=w_t, in_=t_w)
            idx_t = cpool.tile([P, slots_tot // 16], mybir.dt.int16)
            nc.sync.dma_start(out=idx_t, in_=t_idx)
            tloc_t = cpool.tile([P, ncols_pad], mybir.dt.float16)
            nc.sync.dma_start(out=tloc_t, in_=t_tloc)
            gx_t = cpool.tile([P, npad], mybir.dt.float16)
            nc.sync.dma_start(out=gx_t, in_=t_gx)
            ew_t = None
            if t_ew is not None:
                ew_t = cpool.tile([P, ncols_pad], mybir.dt.float16)
                nc.sync.dma_start(out=ew_t, in_=t_ew)
            out_t = cpool.tile([P, npad], mybir.dt.float16)

            state = dict(batch=-1, s_cm=None, pending=None)

            def get_s(cid):
                bi = cid // BATCH
                if bi != state["batch"]:
                    s_cm = spool.tile([P, P, BATCH], mybir.dt.float16,
                                      tag="s")
                    nc.vector.tensor_tensor(
                        out=s_cm[:, :, :], in0=iotacm_t[:, :, :],
                        in1=_bcast_mid(tloc_t[:, bi * BATCH:(bi + 1) * BATCH],
                                       P),
                        op=mybir.AluOpType.is_equal,
                    )
                    if ew_t is not None:
                        s2 = spool.tile([P, P, BATCH], mybir.dt.float16,
                                        tag="s2")
                        nc.vector.tensor_tensor(
                            out=s2[:, :, :], in0=s_cm[:, :, :],
                            in1=_bcast_mid(
                                ew_t[:, bi * BATCH:(bi + 1) * BATCH], P),
                            op=mybir.AluOpType.mult,
                        )
                        s_cm = s2
                    state["batch"] = bi
                    state["s_cm"] = s_cm
                return state["s_cm"][:, :, cid % BATCH]

            def flush_pending():
                if state["pending"] is not None:
                    po, sg, w = state["pending"]
                    nc.vector.tensor_tensor(
                        out=out_t[:, sg:sg + w], in0=po[:, :w],
                        in1=gx_t[:, sg:sg + w], op=mybir.AluOpType.mult)
                    state["pending"] = None

            for g in range(NG):
                nb = min(SBG, NSB - g * SBG)
                g_tiles = []
                for b in range(NBANK):
                    nseg = int(nch[g, b])
                    if nseg == 0:
                        g_tiles.append(None)
                        continue
                    g_t = gpool.tile([P, nseg, P], mybir.dt.float8e3,
                                     tag=f"g{b}")
                    rows = min(BANK, cfg.NV - b * BANK)
                    off = int(seg_base[g, b]) // 16
                    _dma_gather_raw(
                        nc.gpsimd, g_t[:, :, :],
                        t_embs8[b * BANK: b * BANK + rows, 0:P],
                        idx_t[:, off:off + nseg * 8],
                        num_idxs=nseg * P, elem_size=P, elem_step=256,
                    )
                    g_tiles.append(g_t)

                for u0 in range(0, nb, UNIT):
                    uw = min(UNIT, nb - u0)
                    psum_o = psb.tile([P, UNIT * P], mybir.dt.float32,
                                      space="PSUM")
                    for s in range(u0, u0 + uw):
                        cols = sched[g][s]
                        psum_u = psu.tile([P, P], mybir.dt.float32,
                                          space="PSUM")
                        nlast = len(cols) - 1
                        for j, (b, ch, cid) in enumerate(cols):
                            s_ap = get_s(cid)
                            nc.tensor.matmul(
                                out=psum_u[:, :],
                                lhsT=g_tiles[b][:, ch, :],
                                rhs=s_ap,
                                start=(j == 0), stop=(j == nlast),
                            )
                        u16 = xfer.tile([P, P], mybir.dt.float16, tag="u")
                        nc.scalar.activation(
                            out=u16, in_=psum_u,
                            func=mybir.ActivationFunctionType.Copy)
                        q = s - u0
                        nc.tensor.matmul(
                            out=psum_o[:, q * P:(q + 1) * P], lhsT=u16,
                            rhs=w_t, start=True, stop=True,
                            skip_group_check=True)
                    flush_pending()
                    state["pending"] = (psum_o, (g * SBG + u0) * P, uw * P)
            flush_pending()

            nc.sync.dma_start(out=t_out, in_=out_t)
    nc.compile()
    return nc


def _unshard(cfg, res):
    TPC, NSB = cfg.TPC, cfg.NSB
    outs = []
    for c in range(cfg.NCORES):
        o = res.results[c]["out"]                       # [P, NSB*P] fp16
        o = np.asarray(o, np.float32).reshape(P, NSB, P)
        o = o.transpose(1, 0, 2).reshape(NSB * P, P)[:TPC]
        outs.append(o)
    return np.concatenate(outs, axis=0).astype(np.float32)


def kernel(X, embs, W, edge_index, edge_weight):
    cfg = _REAL
    meta, in_maps = _host_prep(cfg, X, embs, W, edge_index, edge_weight)
    nc = _build_program(cfg, meta)
    res = run_bass_kernel_spmd(nc, in_maps, list(range(cfg.NCORES)))
    return _unshard(cfg, res)


# revision 26
# speedup vs baseline: 2.9414x; 1.1321x over previous
"""GCNConv-style message passing kernel for Trainium2, 8 NeuronCores.

Reference semantics:
    deg  = 1 + segment_sum(edge_weight, col)           # self-loop included
    dinv = deg ** -0.5
    h    = embs @ W
    out[t] = (sum_e norm_e * h[src_e] + dinv[t]^2 * h[t]) * X[t],
             norm_e = dinv[src_e] * ew_e * dinv[t]

Device formulation (matmul commutes past the segment sum):
    embs8 = e3m4(SCALE * dinv[:, None] * embs)          (host, fp8 e3m4)
    u[t]  = sum_{e: col=t} ew_e * embs8[src_e]          (self loop folded in
                                                         as an extra edge)
    out[t] = (fp16(u[t]) @ W16) * fp16(dinv[t] * X[t] / SCALE)

Sharding: targets split across 8 cores (12500 each). Edges (incl. self
edges, whose sources point into a per-core virtual row region so the
layout is core-uniform) are grouped by (dest-block-group of SBG x 128
targets, source bank of 32768 rows) into slot segments; within a segment,
per-(dest-block, bank) slot spans are sized max-over-cores so the chunk
schedule is shared SPMD. Source rows are fetched with a raw 128-byte fp8
dma_gather (256B DRAM stride). Selection matrices S[e, t_loc] for BATCH
chunks at a time are built by ONE DVE tensor_tensor is_equal (2x fp16
mode) into a column-major [128, 128, BATCH] tile; chunk j's S is the
stride-BATCH slice [:, :, j], consumed by PE matmuls accumulating PSUM
u^T[cin, t_loc]. Chunks straddling dest-block boundaries emit one masked
S column per covered block. ACT copies PSUM->SBUF as fp16, PE applies W
into 4-block-wide PSUM tiles, DVE multiplies by the gating (deferred one
unit to avoid stalls) and the result is written back once as fp16.
"""

import numpy as np
import ml_dtypes

import concourse.bacc as bacc
import concourse.tile as tile
from concourse import mybir
from concourse.bass import exact_div
from concourse.bass_utils import run_bass_kernel_spmd

P = 128
BATCH = 16            # S columns built per DVE instruction
UNIT = 4              # dest blocks per output psum tile
E3M4 = ml_dtypes.float8_e3m4
SCALE = 4.0
E3M4_MAX = 15.5


class _Cfg:
    def __init__(self, n, n_cores, sb_group):
        self.N = n
        self.NCORES = n_cores
        self.TPC = n // n_cores              # targets per core
        assert self.TPC * n_cores == n
        self.NSB = -(-self.TPC // P)         # dest blocks of 128 per core
        self.BANK = 32768                    # gather bank rows (int16 idx)
        self.NV = n + self.TPC               # rows incl. per-core self region
        self.NBANK = -(-self.NV // self.BANK)
        # group sizes: small first groups (fast pipeline fill), small last
        # groups (short drain), sb_group-wide in the middle
        sizes = [2, 4]
        rem = self.NSB - 2 - 4 - 4
        while rem >= sb_group:
            sizes.append(sb_group)
            rem -= sb_group
        if rem:
            sizes.append(rem)
        sizes += [2, 1, 1]
        assert sum(sizes) == self.NSB
        self.GROUPS = []                     # (start_block, nblocks)
        s0 = 0
        for sz in sizes:
            self.GROUPS.append((s0, sz))
            s0 += sz
        self.NG = len(self.GROUPS)
        self.SBGMAX = max(sz for _, sz in self.GROUPS)
        # block -> (group, local index)
        self.G_OF_SB = np.zeros(self.NSB, np.int64)
        self.SBL_OF_SB = np.zeros(self.NSB, np.int64)
        for g, (st, sz) in enumerate(self.GROUPS):
            self.G_OF_SB[st:st + sz] = g
            self.SBL_OF_SB[st:st + sz] = np.arange(sz)


_REAL = _Cfg(n=100000, n_cores=8, sb_group=8)


def _dma_gather_raw(gp, out_ap, in_ap, idxs_ap, num_idxs, elem_size,
                    elem_step, single_packet=False):
    """bass dma_gather clone (DRAM src, non-transpose) without the
    elem_size%256B restriction; elem_step sets the 256B-unit DRAM stride."""
    assert idxs_ap.dtype == mybir.dt.int16
    assert in_ap.dtype == out_ap.dtype
    assert in_ap.ap[-1][1] == elem_size
    assert out_ap.ap[-1][1] == elem_size
    assert out_ap.ap[0][1] * out_ap.ap[1][1] == ((num_idxs + 127) // 128) * 128
    assert in_ap.ap[0][0] == elem_step
    stride_bytes_256 = exact_div(elem_step * mybir.dt.size(in_ap.dtype), 256)
    assert 0 < stride_bytes_256 < 256
    _in_ap = gp.lower_ap_dma(in_ap, for_custom_bir_dma=True)
    _idxs_ap = gp.lower_ap(idxs_ap)
    _out_ap = gp.lower_ap(out_ap)
    return gp.add_instruction(
        mybir.InstDMAGatherAnt(
            name=gp.bass.get_next_instruction_name(),
            ins=[*_in_ap, _idxs_ap, gp.lower_val_access(gp.to_reg(num_idxs))],
            outs=[_out_ap],
            transpose=False,
            num_idxs=num_idxs,
            elem_size=elem_size,
            stride_bytes_256=stride_bytes_256,
            gen_mode=0,
            single_packet=single_packet,
            queue_num=0,
            sbuf_tokens_per_rank=0,
            sbuf_free_dim_per_rank=0,
            sbuf_free_dim_pad_per_rank=0,
            sbuf_byte_offset=0,
        )
    )


def _bcast_mid(ap, rep):
    """[P, k] AP -> [P, rep, k] with a stride-0 middle dim."""
    cls = type(ap)
    new = [list(ap.ap[0]), [0, rep], list(ap.ap[1])]
    return cls(ap.tensor, ap.offset, new)


def _host_prep(cfg, X, embs, W, edge_index, edge_weight):
    """Build fp8 embs table, slot layout, chunk schedule, per-core arrays."""
    N, TPC, NSB, BANK, NBANK = cfg.N, cfg.TPC, cfg.NSB, cfg.BANK, cfg.NBANK
    NC, SBG, NG = cfg.NCORES, cfg.SBGMAX, cfg.NG

    src = np.asarray(edge_index[0], dtype=np.int64)
    col = np.asarray(edge_index[1], dtype=np.int64)
    ew = np.asarray(edge_weight, dtype=np.float32)
    ew_ones = bool(np.all(ew == 1.0))

    # self loops as ordinary edges; their source points into a per-core
    # virtual row region [N, N+TPC) so the (block, bank) slot layout is
    # identical across cores.
    loop = np.arange(N, dtype=np.int64)
    src_a = np.concatenate([src, N + (loop % TPC)])
    col_a = np.concatenate([col, loop])
    ew_a = np.concatenate([ew, np.ones(N, np.float32)])

    deg = 1.0 + np.bincount(col, weights=ew.astype(np.float64), minlength=N)
    dinv = (1.0 / np.sqrt(deg)).astype(np.float32)

    embs_s = dinv[:, None] * np.asarray(embs, np.float32)
    embs8 = np.zeros((cfg.NV, 256), E3M4)
    embs8[:N, :P] = np.clip(embs_s * SCALE, -E3M4_MAX, E3M4_MAX).astype(E3M4)
    gx = (dinv[:, None] * np.asarray(X, np.float32)) * (1.0 / SCALE)

    # bucket keys
    core = col_a // TPC
    sb = (col_a % TPC) // P                  # 0..NSB-1
    grp = cfg.G_OF_SB[sb]
    sbl = cfg.SBL_OF_SB[sb]                  # block local to group
    bank = src_a // BANK

    key = ((core * NG + grp) * NBANK + bank) * SBG + sbl
    nkey = NC * NG * NBANK * SBG
    counts = np.bincount(key, minlength=nkey).reshape(NC, NG, NBANK, SBG)
    M = counts.max(axis=0)                   # shared span sizes [NG,NBANK,SBG]

    # shared slot layout: segments (g,b) in order, blocks in order inside,
    # each segment padded to a 128 multiple.
    blk_off = np.zeros((NG, NBANK, SBG), np.int64)   # block span start
    seg_base = np.zeros((NG, NBANK), np.int64)       # segment slot base
    nch = np.zeros((NG, NBANK), np.int64)            # chunks per segment
    pos = 0
    for g in range(NG):
        nb = cfg.GROUPS[g][1]
        for b in range(NBANK):
            seg_base[g, b] = pos
            off = 0
            for s in range(nb):
                blk_off[g, b, s] = off
                off += M[g, b, s]
            nch[g, b] = -(-off // P)
            pos += nch[g, b] * P
    slots_tot = pos

    # chunk/block overlap -> tloc columns, in device processing order
    colid = {}
    sched = []                               # [g][s_local] -> [(b, ch, col)]
    ncols = 0
    for g in range(NG):
        nb = cfg.GROUPS[g][1]
        gsched = []
        for s in range(nb):
            lst = []
            for b in range(NBANK):
                lo = blk_off[g, b, s]
                hi = lo + M[g, b, s]
                if hi == lo:
                    continue
                c0, c1 = lo // P, (hi - 1) // P
                for c in range(c0, c1 + 1):
                    colid[(g, b, c, s)] = ncols
                    lst.append((b, int(c), ncols))
                    ncols += 1
            assert lst, f"block {g},{s} has no slots"
            gsched.append(lst)
        sched.append(gsched)
    ncols_pad = -(-ncols // BATCH) * BATCH

    # per-edge destination slots
    cnt_flat = counts.reshape(-1)
    order = np.argsort(key, kind="stable")
    starts = np.zeros_like(cnt_flat)
    np.cumsum(cnt_flat[:-1], out=starts[1:])
    k_sorted = key[order]
    rank = np.arange(len(order)) - starts[k_sorted]
    g_s, b_s = grp[order], bank[order]
    sbl_s, core_s = sbl[order], core[order]
    dest = (seg_base[g_s, b_s] + blk_off[g_s, b_s, sbl_s] + rank)
    chunk_s = (dest - seg_base[g_s, b_s]) // P    # segment-local chunk
    lane_s = dest % P
    tl_s = (col_a[order] % TPC % P).astype(np.float32)
    src_l = (src_a[order] - b_s * BANK).astype(np.int16)
    ew_s = ew_a[order]

    # column index per edge (vectorized via dict -> array)
    mchunk = slots_tot // P + 1
    ckey = ((g_s * NBANK + b_s) * mchunk + chunk_s) * SBG + sbl_s
    uk, inv = np.unique(ckey, return_inverse=True)
    uk_col = np.empty(len(uk), np.int64)
    for i, k in enumerate(uk):
        sblk = k % SBG
        k //= SBG
        ch = k % mchunk
        k //= mchunk
        b = k % NBANK
        g = k // NBANK
        uk_col[i] = colid[(g, b, ch, sblk)]
    col_e = uk_col[inv]

    IDX = np.zeros((NC, slots_tot), np.int16)
    TLOC = np.full((NC, P, ncols_pad), -100.0, np.float16)
    IDX[core_s, dest] = src_l
    TLOC[core_s, lane_s, col_e] = tl_s
    EWC = None
    if not ew_ones:
        EWC = np.ones((NC, P, ncols_pad), np.float16)
        EWC[core_s, lane_s, col_e] = ew_s

    # pack gather indices wrap-16, replicate to 128 partitions
    idx_packed = IDX.reshape(NC, slots_tot // 16, 16).transpose(0, 2, 1)
    idx_all = np.tile(idx_packed, (1, 8, 1)).astype(np.int16)

    # partition-major gx layout: pm[p, s*128 + c] = row (s*128+p)
    npad = NSB * P
    gx_pm = np.zeros((NC, P, npad), np.float16)
    for c in range(NC):
        gxc = gx[c * TPC:(c + 1) * TPC]
        gxc = np.concatenate(
            [gxc, np.zeros((npad - TPC, P), np.float32)], axis=0)
        gx_pm[c] = gxc.reshape(NSB, P, P).transpose(1, 0, 2).reshape(
            P, npad).astype(np.float16)

    # iotacm[p, t*BATCH + j] = t  (column-major S layout constant)
    iotacm = np.repeat(np.arange(P, dtype=np.float16), BATCH)[None, :]
    iotacm = np.tile(iotacm, (P, 1))
    w16 = np.asarray(W, np.float16)

    meta = dict(sched=sched, nch=nch, seg_base=seg_base, slots_tot=slots_tot,
                ncols=ncols, ncols_pad=ncols_pad, ew_ones=ew_ones)
    in_maps = []
    for c in range(NC):
        e8 = embs8.copy()
        e8[N:N + TPC] = embs8[c * TPC:(c + 1) * TPC]
        m = dict(
            embs8=e8,
            w16=w16,
            gx=np.ascontiguousarray(gx_pm[c]),
            idxall=np.ascontiguousarray(idx_all[c]),
            tlocall=np.ascontiguousarray(TLOC[c].reshape(P, ncols_pad)),
            iotacm=iotacm,
        )
        if EWC is not None:
            m["ewall"] = np.ascontiguousarray(EWC[c].reshape(P, ncols_pad))
        in_maps.append(m)
    return meta, in_maps


def _build_program(cfg, meta):
    N, TPC, NSB, BANK, NBANK = cfg.N, cfg.TPC, cfg.NSB, cfg.BANK, cfg.NBANK
    NG = cfg.NG
    sched, nch, seg_base = meta["sched"], meta["nch"], meta["seg_base"]
    slots_tot, ncols_pad, ew_ones = (meta["slots_tot"], meta["ncols_pad"],
                                     meta["ew_ones"])
    npad = NSB * P

    nc = bacc.Bacc("TRN2", target_bir_lowering=False, debug=False,
                   num_devices=cfg.NCORES)
    t_embs8 = nc.dram_tensor("embs8", [cfg.NV, 256], mybir.dt.float8e3,
                             kind="ExternalInput").ap()
    t_w = nc.dram_tensor("w16", [P, P], mybir.dt.float16,
                         kind="ExternalInput").ap()
    t_gx = nc.dram_tensor("gx", [P, npad], mybir.dt.float16,
                          kind="ExternalInput").ap()
    t_idx = nc.dram_tensor("idxall", [P, slots_tot // 16], mybir.dt.int16,
                           kind="ExternalInput").ap()
    t_tloc = nc.dram_tensor("tlocall", [P, ncols_pad], mybir.dt.float16,
                            kind="ExternalInput").ap()
    t_iotacm = nc.dram_tensor("iotacm", [P, P * BATCH], mybir.dt.float16,
                              kind="ExternalInput").ap()
    t_ew = None
    if not ew_ones:
        t_ew = nc.dram_tensor("ewall", [P, ncols_pad], mybir.dt.float16,
                              kind="ExternalInput").ap()
    t_out = nc.dram_tensor("out", [P, npad], mybir.dt.float16,
                           kind="ExternalOutput").ap()

    with tile.TileContext(nc) as tc:
        with tc.tile_pool(name="const", bufs=1) as cpool, \
             tc.tile_pool(name="gpool", bufs=3) as gpool, \
             tc.tile_pool(name="spool", bufs=4) as spool, \
             tc.tile_pool(name="xfer", bufs=4) as xfer, \
             tc.tile_pool(name="psu", bufs=4, space="PSUM") as psu, \
             tc.tile_pool(name="psb", bufs=3, space="PSUM") as psb:

            tloc_t = cpool.tile([P, ncols_pad], mybir.dt.float16)
            nc.sync.dma_start(out=tloc_t, in_=t_tloc)
            iotacm_t = cpool.tile([P, P, BATCH], mybir.dt.float16)
            nc.sync.dma_start(out=iotacm_t, in_=t_iotacm)
            w_t = cpool.tile([P, P], mybir.dt.float16)
            nc.sync.dma_start(out=w_t, in_=t_w)
            idx_t = cpool.tile([P, slots_tot // 16], mybir.dt.int16)
            tloc_loaded = True
            ew_t = None
            if t_ew is not None:
                ew_t = cpool.tile([P, ncols_pad], mybir.dt.float16)
                nc.sync.dma_start(out=ew_t, in_=t_ew)
            gx_t = cpool.tile([P, npad], mybir.dt.float16)
            out_t = cpool.tile([P, npad], mybir.dt.float16)
            gx_loaded = False

            state = dict(batch=-1, s_cm=None, pending=None, wrote=0)

            def get_s(cid):
                bi = cid // BATCH
                if bi != state["batch"]:
                    s_cm = spool.tile([P, P, BATCH], mybir.dt.float16,
                                      tag="s")
                    nc.vector.tensor_tensor(
                        out=s_cm[:, :, :], in0=iotacm_t[:, :, :],
                        in1=_bcast_mid(tloc_t[:, bi * BATCH:(bi + 1) * BATCH],
                                       P),
                        op=mybir.AluOpType.is_equal,
                    )
                    if ew_t is not None:
                        s2 = spool.tile([P, P, BATCH], mybir.dt.float16,
                                        tag="s2")
                        nc.vector.tensor_tensor(
                            out=s2[:, :, :], in0=s_cm[:, :, :],
                            in1=_bcast_mid(
                                ew_t[:, bi * BATCH:(bi + 1) * BATCH], P),
                            op=mybir.AluOpType.mult,
                        )
                        s_cm = s2
                    state["batch"] = bi
                    state["s_cm"] = s_cm
                return state["s_cm"][:, :, cid % BATCH]

            def flush_pending():
                if state["pending"] is None:
                    return
                po, sg, w, gtag = state["pending"]
                nc.vector.tensor_tensor(
                    out=out_t[:, sg:sg + w], in0=po[:, :w],
                    in1=gx_t[:, sg:sg + w], op=mybir.AluOpType.mult)
                state["pending"] = None
                while state["wrote"] < gtag:     # groups < gtag are complete
                    h = state["wrote"]
                    st_h, nb_h = cfg.GROUPS[h]
                    c0, c1 = st_h * P, (st_h + nb_h) * P
                    nc.sync.dma_start(out=t_out[:, c0:c1],
                                      in_=out_t[:, c0:c1])
                    state["wrote"] = h + 1

            for g in range(NG):
                st_g, nb = cfg.GROUPS[g]
                g_tiles = []
                for b in range(NBANK):
                    nseg = int(nch[g, b])
                    if nseg == 0:
                        g_tiles.append(None)
                        continue
                    g_t = gpool.tile([P, nseg, P], mybir.dt.float8e3,
                                     tag=f"g{b}")
                    rows = min(BANK, cfg.NV - b * BANK)
                    off = int(seg_base[g, b]) // 16
                    nc.sync.dma_start(out=idx_t[:, off:off + nseg * 8],
                                      in_=t_idx[:, off:off + nseg * 8])
                    _dma_gather_raw(
                        nc.gpsimd, g_t[:, :, :],
                        t_embs8[b * BANK: b * BANK + rows, 0:P],
                        idx_t[:, off:off + nseg * 8],
                        num_idxs=nseg * P, elem_size=P, elem_step=256,
                    )
                    g_tiles.append(g_t)
                if not gx_loaded:
                    # deferred so the first gathers go out first
                    nc.sync.dma_start(out=gx_t, in_=t_gx)
                    gx_loaded = True

                for u0 in range(0, nb, UNIT):
                    uw = min(UNIT, nb - u0)
                    psum_o = psb.tile([P, UNIT * P], mybir.dt.float32,
                                      space="PSUM")
                    for s in range(u0, u0 + uw):
                        cols = sched[g][s]
                        psum_u = psu.tile([P, P], mybir.dt.float32,
                                          space="PSUM")
                        nlast = len(cols) - 1
                        for j, (b, ch, cid) in enumerate(cols):
                            s_ap = get_s(cid)
                            nc.tensor.matmul(
                                out=psum_u[:, :],
                                lhsT=g_tiles[b][:, ch, :],
                                rhs=s_ap,
                                start=(j == 0), stop=(j == nlast),
                            )
                        u16 = xfer.tile([P, P], mybir.dt.float16, tag="u")
                        nc.scalar.activation(
                            out=u16, in_=psum_u,
                            func=mybir.ActivationFunctionType.Copy)
                        q = s - u0
                        nc.tensor.matmul(
                            out=psum_o[:, q * P:(q + 1) * P], lhsT=u16,
                            rhs=w_t, start=True, stop=True,
                            skip_group_check=True)
                    flush_pending()
                    state["pending"] = (psum_o, (st_g + u0) * P, uw * P, g)
            flush_pending()
            while state["wrote"] < NG:
                h = state["wrote"]
                st_h, nb_h = cfg.GROUPS[h]
                c0, c1 = st_h * P, (st_h + nb_h) * P
                nc.sync.dma_start(out=t_out[:, c0:c1], in_=out_t[:, c0:c1])
                state["wrote"] = h + 1
    nc.compile()
    return nc


def _unshard(cfg, res):
    TPC, NSB = cfg.TPC, cfg.NSB
    outs = []
    for c in range(cfg.NCORES):
        o = res.results[c]["out"]                       # [P, NSB*P] fp16
        o = np.asarray(o, np.float32).reshape(P, NSB, P)
        o = o.transpose(1, 0, 2).reshape(NSB * P, P)[:TPC]
        outs.append(o)
    return np.concatenate(outs, axis=0).astype(np.float32)


def kernel(X, embs, W, edge_index, edge_weight):
    cfg = _REAL
    meta, in_maps = _host_prep(cfg, X, embs, W, edge_index, edge_weight)
    nc = _build_program(cfg, meta)
    res = run_bass_kernel_spmd(nc, in_maps, list(range(cfg.NCORES)))
    return _unshard(cfg, res)


# revision 36
# speedup vs baseline: 3.0261x; 1.0288x over previous
"""GCNConv-style message passing kernel for Trainium2, 8 NeuronCores.

Reference semantics:
    deg  = 1 + segment_sum(edge_weight, col)           # self-loop included
    dinv = deg ** -0.5
    h    = embs @ W
    out[t] = (sum_e norm_e * h[src_e] + dinv[t]^2 * h[t]) * X[t],
             norm_e = dinv[src_e] * ew_e * dinv[t]

Device formulation (matmul commutes past the segment sum):
    embs8 = e3m4(SCALE * dinv[:, None] * embs)          (host, fp8 e3m4)
    u[t]  = sum_{e: col=t} ew_e * embs8[src_e]          (self loop folded in
                                                         as an extra edge)
    out[t] = (fp16(u[t]) @ W16) * fp16(dinv[t] * X[t] / SCALE)

Sharding: targets split across 8 cores (12500 each). Edges (incl. self
edges, whose sources point into a per-core virtual row region so the
layout is core-uniform) are grouped by (dest-block-group of SBG x 128
targets, source bank of 32768 rows) into slot segments; within a segment,
per-(dest-block, bank) slot spans are sized max-over-cores so the chunk
schedule is shared SPMD. Source rows are fetched with a raw 128-byte fp8
dma_gather (256B DRAM stride). Selection matrices S[e, t_loc] for BATCH
chunks at a time are built by ONE DVE tensor_tensor is_equal (2x fp16
mode) into a column-major [128, 128, BATCH] tile; chunk j's S is the
stride-BATCH slice [:, :, j], consumed by PE matmuls accumulating PSUM
u^T[cin, t_loc]. Chunks straddling dest-block boundaries emit one masked
S column per covered block. ACT copies PSUM->SBUF as fp16, PE applies W
into 4-block-wide PSUM tiles, DVE multiplies by the gating (deferred one
unit to avoid stalls) and the result is written back once as fp16.
"""

import numpy as np
import ml_dtypes

import concourse.bacc as bacc
import concourse.tile as tile
from concourse import mybir
from concourse.bass import exact_div
from concourse.bass_utils import run_bass_kernel_spmd

P = 128
BATCH = 16            # S columns built per DVE instruction
UNIT = 4              # dest blocks per output psum tile
E3M4 = ml_dtypes.float8_e3m4
SCALE = 4.0
E3M4_MAX = 15.5


class _Cfg:
    def __init__(self, n, n_cores, sb_group):
        self.N = n
        self.NCORES = n_cores
        self.TPC = n // n_cores              # targets per core
        assert self.TPC * n_cores == n
        self.NSB = -(-self.TPC // P)         # dest blocks of 128 per core
        self.BANK = 32768                    # gather bank rows (int16 idx)
        self.NV = n + self.TPC               # rows incl. per-core self region
        self.NBANK = -(-self.NV // self.BANK)
        # taper the last groups so the end-of-pipeline drain is short
        tail = [4, 3, 2, 1]
        sizes = []
        rem = self.NSB - sum(tail)
        if rem < sb_group:
            tail, rem = [], self.NSB
        while rem >= sb_group:
            sizes.append(sb_group)
            rem -= sb_group
        if rem:
            sizes.append(rem)
        sizes += tail
        assert sum(sizes) == self.NSB
        self.GROUPS = []                     # (start_block, nblocks)
        s0 = 0
        for sz in sizes:
            self.GROUPS.append((s0, sz))
            s0 += sz
        self.NG = len(self.GROUPS)
        self.SBGMAX = max(sz for _, sz in self.GROUPS)
        # block -> (group, local index)
        self.G_OF_SB = np.zeros(self.NSB, np.int64)
        self.SBL_OF_SB = np.zeros(self.NSB, np.int64)
        for g, (st, sz) in enumerate(self.GROUPS):
            self.G_OF_SB[st:st + sz] = g
            self.SBL_OF_SB[st:st + sz] = np.arange(sz)


_REAL = _Cfg(n=100000, n_cores=8, sb_group=8)


def _dma_gather_raw(gp, out_ap, in_ap, idxs_ap, num_idxs, elem_size,
                    elem_step, single_packet=False):
    """bass dma_gather clone (DRAM src, non-transpose) without the
    elem_size%256B restriction; elem_step sets the 256B-unit DRAM stride."""
    assert idxs_ap.dtype == mybir.dt.int16
    assert in_ap.dtype == out_ap.dtype
    assert in_ap.ap[-1][1] == elem_size
    assert out_ap.ap[-1][1] == elem_size
    assert out_ap.ap[0][1] * out_ap.ap[1][1] == ((num_idxs + 127) // 128) * 128
    assert in_ap.ap[0][0] == elem_step
    stride_bytes_256 = exact_div(elem_step * mybir.dt.size(in_ap.dtype), 256)
    assert 0 < stride_bytes_256 < 256
    _in_ap = gp.lower_ap_dma(in_ap, for_custom_bir_dma=True)
    _idxs_ap = gp.lower_ap(idxs_ap)
    _out_ap = gp.lower_ap(out_ap)
    return gp.add_instruction(
        mybir.InstDMAGatherAnt(
            name=gp.bass.get_next_instruction_name(),
            ins=[*_in_ap, _idxs_ap, gp.lower_val_access(gp.to_reg(num_idxs))],
            outs=[_out_ap],
            transpose=False,
            num_idxs=num_idxs,
            elem_size=elem_size,
            stride_bytes_256=stride_bytes_256,
            gen_mode=0,
            single_packet=single_packet,
            queue_num=0,
            sbuf_tokens_per_rank=0,
            sbuf_free_dim_per_rank=0,
            sbuf_free_dim_pad_per_rank=0,
            sbuf_byte_offset=0,
        )
    )


def _bcast_mid(ap, rep):
    """[P, k] AP -> [P, rep, k] with a stride-0 middle dim."""
    cls = type(ap)
    new = [list(ap.ap[0]), [0, rep], list(ap.ap[1])]
    return cls(ap.tensor, ap.offset, new)


def _balance_perm(cfg, src, col):
    """Greedy target -> virtual-id assignment equalizing per-(block
    position, source bank) in-degree across cores, which shrinks the
    max-over-cores slot padding. perm[t] = virtual id."""
    N, TPC, NSB, BANK, NBANK = cfg.N, cfg.TPC, cfg.NSB, cfg.BANK, cfg.NBANK
    NC = cfg.NCORES
    dkey = col * NBANK + src // BANK
    d = np.bincount(dkey, minlength=N * NBANK).reshape(N, NBANK)
    tot = d.sum(1)
    order = np.argsort(-tot, kind="stable")
    perm = np.empty(N, np.int64)
    pos = 0
    for k in range(NSB):
        blkN = min(P, TPC - k * P)
        cand = order[pos: pos + blkN * NC]
        pos += blkN * NC
        cnt = np.zeros(NC, np.int64)
        R = np.zeros((NC, NBANK), np.int64)
        full = np.zeros(NC, bool)
        for t in cand:
            cost = (R + d[t]).max(axis=1) + np.where(full, 1 << 30, 0)
            c = int(np.argmin(cost))
            perm[t] = c * TPC + k * P + cnt[c]
            R[c] += d[t]
            cnt[c] += 1
            if cnt[c] == blkN:
                full[c] = True
    return perm


def _host_prep(cfg, X, embs, W, edge_index, edge_weight):
    """Build fp8 embs table, slot layout, chunk schedule, per-core arrays."""
    N, TPC, NSB, BANK, NBANK = cfg.N, cfg.TPC, cfg.NSB, cfg.BANK, cfg.NBANK
    NC, SBG, NG = cfg.NCORES, cfg.SBGMAX, cfg.NG

    src = np.asarray(edge_index[0], dtype=np.int64)
    col = np.asarray(edge_index[1], dtype=np.int64)
    ew = np.asarray(edge_weight, dtype=np.float32)
    ew_ones = bool(np.all(ew == 1.0))

    perm = _balance_perm(cfg, src, col)
    inv = np.empty(N, np.int64)
    inv[perm] = np.arange(N)

    # self loops as ordinary edges; their source points into a per-core
    # virtual row region [N, N+TPC) so the (block, bank) slot layout is
    # identical across cores. Targets are remapped through perm.
    loop = np.arange(N, dtype=np.int64)
    src_a = np.concatenate([src, N + (perm[loop] % TPC)])
    col_a = np.concatenate([perm[col], perm[loop]])
    ew_a = np.concatenate([ew, np.ones(N, np.float32)])

    deg = 1.0 + np.bincount(col, weights=ew.astype(np.float64), minlength=N)
    dinv = (1.0 / np.sqrt(deg)).astype(np.float32)

    embs_s = dinv[:, None] * np.asarray(embs, np.float32)
    embs8 = np.zeros((cfg.NV, 256), E3M4)
    embs8[:N, :P] = np.clip(embs_s * SCALE, -E3M4_MAX, E3M4_MAX).astype(E3M4)
    gx = (dinv[:, None] * np.asarray(X, np.float32)) * (1.0 / SCALE)

    # bucket keys
    core = col_a // TPC
    sb = (col_a % TPC) // P                  # 0..NSB-1
    grp = cfg.G_OF_SB[sb]
    sbl = cfg.SBL_OF_SB[sb]                  # block local to group
    bank = src_a // BANK

    key = ((core * NG + grp) * NBANK + bank) * SBG + sbl
    nkey = NC * NG * NBANK * SBG
    counts = np.bincount(key, minlength=nkey).reshape(NC, NG, NBANK, SBG)
    M = counts.max(axis=0)                   # shared span sizes [NG,NBANK,SBG]

    # shared slot layout: segments (g,b) in order, blocks in order inside,
    # each segment padded to a 128 multiple.
    blk_off = np.zeros((NG, NBANK, SBG), np.int64)   # block span start
    seg_base = np.zeros((NG, NBANK), np.int64)       # segment slot base
    nch = np.zeros((NG, NBANK), np.int64)            # chunks per segment
    pos = 0
    for g in range(NG):
        nb = cfg.GROUPS[g][1]
        for b in range(NBANK):
            seg_base[g, b] = pos
            off = 0
            for s in range(nb):
                blk_off[g, b, s] = off
                off += M[g, b, s]
            nch[g, b] = -(-off // P)
            pos += nch[g, b] * P
    slots_tot = pos

    # chunk/block overlap -> tloc columns, in device processing order
    colid = {}
    sched = []                               # [g][s_local] -> [(b, ch, col)]
    ncols = 0
    for g in range(NG):
        nb = cfg.GROUPS[g][1]
        gsched = []
        for s in range(nb):
            lst = []
            for b in range(NBANK):
                lo = blk_off[g, b, s]
                hi = lo + M[g, b, s]
                if hi == lo:
                    continue
                c0, c1 = lo // P, (hi - 1) // P
                for c in range(c0, c1 + 1):
                    colid[(g, b, c, s)] = ncols
                    lst.append((b, int(c), ncols))
                    ncols += 1
            assert lst, f"block {g},{s} has no slots"
            gsched.append(lst)
        sched.append(gsched)
    ncols_pad = -(-ncols // BATCH) * BATCH

    # per-edge destination slots
    cnt_flat = counts.reshape(-1)
    order = np.argsort(key, kind="stable")
    starts = np.zeros_like(cnt_flat)
    np.cumsum(cnt_flat[:-1], out=starts[1:])
    k_sorted = key[order]
    rank = np.arange(len(order)) - starts[k_sorted]
    g_s, b_s = grp[order], bank[order]
    sbl_s, core_s = sbl[order], core[order]
    dest = (seg_base[g_s, b_s] + blk_off[g_s, b_s, sbl_s] + rank)
    chunk_s = (dest - seg_base[g_s, b_s]) // P    # segment-local chunk
    lane_s = dest % P
    tl_s = (col_a[order] % TPC % P).astype(np.float32)
    src_l = (src_a[order] - b_s * BANK).astype(np.int16)
    ew_s = ew_a[order]

    # column index per edge (vectorized via dict -> array)
    mchunk = slots_tot // P + 1
    ckey = ((g_s * NBANK + b_s) * mchunk + chunk_s) * SBG + sbl_s
    uk, uinv = np.unique(ckey, return_inverse=True)
    uk_col = np.empty(len(uk), np.int64)
    for i, k in enumerate(uk):
        sblk = k % SBG
        k //= SBG
        ch = k % mchunk
        k //= mchunk
        b = k % NBANK
        g = k // NBANK
        uk_col[i] = colid[(g, b, ch, sblk)]
    col_e = uk_col[uinv]

    IDX = np.zeros((NC, slots_tot), np.int16)
    TLOC = np.full((NC, P, ncols_pad), -100.0, np.float16)
    IDX[core_s, dest] = src_l
    TLOC[core_s, lane_s, col_e] = tl_s
    EWC = None
    if not ew_ones:
        EWC = np.ones((NC, P, ncols_pad), np.float16)
        EWC[core_s, lane_s, col_e] = ew_s

    # pack gather indices wrap-16, replicate to 128 partitions
    idx_packed = IDX.reshape(NC, slots_tot // 16, 16).transpose(0, 2, 1)
    idx_all = np.tile(idx_packed, (1, 8, 1)).astype(np.int16)

    # partition-major gx layout in virtual row order
    npad = NSB * P
    gx_v = gx[inv]
    gx_pm = np.zeros((NC, P, npad), np.float16)
    for c in range(NC):
        gxc = gx_v[c * TPC:(c + 1) * TPC]
        gxc = np.concatenate(
            [gxc, np.zeros((npad - TPC, P), np.float32)], axis=0)
        gx_pm[c] = gxc.reshape(NSB, P, P).transpose(1, 0, 2).reshape(
            P, npad).astype(np.float16)

    # iotacm[p, t*BATCH + j] = t  (column-major S layout constant)
    iotacm = np.repeat(np.arange(P, dtype=np.float16), BATCH)[None, :]
    iotacm = np.tile(iotacm, (P, 1))
    w16 = np.asarray(W, np.float16)

    meta = dict(sched=sched, nch=nch, seg_base=seg_base, slots_tot=slots_tot,
                ncols=ncols, ncols_pad=ncols_pad, ew_ones=ew_ones, perm=perm)
    in_maps = []
    for c in range(NC):
        e8 = embs8.copy()
        e8[N:N + TPC] = embs8[inv[c * TPC:(c + 1) * TPC]]
        m = dict(
            embs8=e8,
            w16=w16,
            gx=np.ascontiguousarray(gx_pm[c]),
            idxall=np.ascontiguousarray(idx_all[c]),
            tlocall=np.ascontiguousarray(TLOC[c].reshape(P, ncols_pad)),
            iotacm=iotacm,
        )
        if EWC is not None:
            m["ewall"] = np.ascontiguousarray(EWC[c].reshape(P, ncols_pad))
        in_maps.append(m)
    return meta, in_maps


def _build_program(cfg, meta):
    N, TPC, NSB, BANK, NBANK = cfg.N, cfg.TPC, cfg.NSB, cfg.BANK, cfg.NBANK
    NG = cfg.NG
    sched, nch, seg_base = meta["sched"], meta["nch"], meta["seg_base"]
    slots_tot, ncols_pad, ew_ones = (meta["slots_tot"], meta["ncols_pad"],
                                     meta["ew_ones"])
    npad = NSB * P

    nc = bacc.Bacc("TRN2", target_bir_lowering=False, debug=False,
                   num_devices=cfg.NCORES)
    t_embs8 = nc.dram_tensor("embs8", [cfg.NV, 256], mybir.dt.float8e3,
                             kind="ExternalInput").ap()
    t_w = nc.dram_tensor("w16", [P, P], mybir.dt.float16,
                         kind="ExternalInput").ap()
    t_gx = nc.dram_tensor("gx", [P, npad], mybir.dt.float16,
                          kind="ExternalInput").ap()
    t_idx = nc.dram_tensor("idxall", [P, slots_tot // 16], mybir.dt.int16,
                           kind="ExternalInput").ap()
    t_tloc = nc.dram_tensor("tlocall", [P, ncols_pad], mybir.dt.float16,
                            kind="ExternalInput").ap()
    t_iotacm = nc.dram_tensor("iotacm", [P, P * BATCH], mybir.dt.float16,
                              kind="ExternalInput").ap()
    t_ew = None
    if not ew_ones:
        t_ew = nc.dram_tensor("ewall", [P, ncols_pad], mybir.dt.float16,
                              kind="ExternalInput").ap()
    t_out = nc.dram_tensor("out", [P, npad], mybir.dt.float16,
                           kind="ExternalOutput").ap()

    with tile.TileContext(nc) as tc:
        with tc.tile_pool(name="const", bufs=1) as cpool, \
             tc.tile_pool(name="gpool", bufs=4) as gpool, \
             tc.tile_pool(name="spool", bufs=4) as spool, \
             tc.tile_pool(name="xfer", bufs=4) as xfer, \
             tc.tile_pool(name="psu", bufs=4, space="PSUM") as psu, \
             tc.tile_pool(name="psb", bufs=3, space="PSUM") as psb:

            tloc_t = cpool.tile([P, ncols_pad], mybir.dt.float16)
            nc.sync.dma_start(out=tloc_t, in_=t_tloc)
            iotacm_t = cpool.tile([P, P, BATCH], mybir.dt.float16)
            nc.sync.dma_start(out=iotacm_t, in_=t_iotacm)
            w_t = cpool.tile([P, P], mybir.dt.float16)
            nc.sync.dma_start(out=w_t, in_=t_w)
            idx_t = cpool.tile([P, slots_tot // 16], mybir.dt.int16)
            tloc_loaded = True
            ew_t = None
            if t_ew is not None:
                ew_t = cpool.tile([P, ncols_pad], mybir.dt.float16)
                nc.sync.dma_start(out=ew_t, in_=t_ew)
            gx_t = cpool.tile([P, npad], mybir.dt.float16)
            out_t = cpool.tile([P, npad], mybir.dt.float16)
            gx_loaded = False

            state = dict(batch=-1, s_cm=None, pending=None, wrote=0)

            def get_s(cid):
                bi = cid // BATCH
                if bi != state["batch"]:
                    s_cm = spool.tile([P, P, BATCH], mybir.dt.float16,
                                      tag="s")
                    nc.vector.tensor_tensor(
                        out=s_cm[:, :, :], in0=iotacm_t[:, :, :],
                        in1=_bcast_mid(tloc_t[:, bi * BATCH:(bi + 1) * BATCH],
                                       P),
                        op=mybir.AluOpType.is_equal,
                    )
                    if ew_t is not None:
                        s2 = spool.tile([P, P, BATCH], mybir.dt.float16,
                                        tag="s2")
                        nc.vector.tensor_tensor(
                            out=s2[:, :, :], in0=s_cm[:, :, :],
                            in1=_bcast_mid(
                                ew_t[:, bi * BATCH:(bi + 1) * BATCH], P),
                            op=mybir.AluOpType.mult,
                        )
                        s_cm = s2
                    state["batch"] = bi
                    state["s_cm"] = s_cm
                return state["s_cm"][:, :, cid % BATCH]

            def flush_pending():
                if state["pending"] is None:
                    return
                po, sg, w, gtag = state["pending"]
                nc.vector.tensor_tensor(
                    out=out_t[:, sg:sg + w], in0=po[:, :w],
                    in1=gx_t[:, sg:sg + w], op=mybir.AluOpType.mult)
                state["pending"] = None
                while state["wrote"] < gtag:     # groups < gtag are complete
                    h = state["wrote"]
                    st_h, nb_h = cfg.GROUPS[h]
                    c0, c1 = st_h * P, (st_h + nb_h) * P
                    nc.sync.dma_start(out=t_out[:, c0:c1],
                                      in_=out_t[:, c0:c1])
                    state["wrote"] = h + 1

            for g in range(NG):
                st_g, nb = cfg.GROUPS[g]
                g_tiles = []
                for b in range(NBANK):
                    nseg = int(nch[g, b])
                    if nseg == 0:
                        g_tiles.append(None)
                        continue
                    g_t = gpool.tile([P, nseg, P], mybir.dt.float8e3,
                                     tag=f"g{b}")
                    rows = min(BANK, cfg.NV - b * BANK)
                    off = int(seg_base[g, b]) // 16
                    nc.sync.dma_start(out=idx_t[:, off:off + nseg * 8],
                                      in_=t_idx[:, off:off + nseg * 8])
                    _dma_gather_raw(
                        nc.gpsimd, g_t[:, :, :],
                        t_embs8[b * BANK: b * BANK + rows, 0:P],
                        idx_t[:, off:off + nseg * 8],
                        num_idxs=nseg * P, elem_size=P, elem_step=256,
                    )
                    g_tiles.append(g_t)
                if not gx_loaded:
                    # deferred so the first gathers go out first
                    nc.sync.dma_start(out=gx_t, in_=t_gx)
                    gx_loaded = True

                for u0 in range(0, nb, UNIT):
                    uw = min(UNIT, nb - u0)
                    psum_o = psb.tile([P, UNIT * P], mybir.dt.float32,
                                      space="PSUM")
                    for s in range(u0, u0 + uw):
                        cols = sched[g][s]
                        psum_u = psu.tile([P, P], mybir.dt.float32,
                                          space="PSUM")
                        nlast = len(cols) - 1
                        for j, (b, ch, cid) in enumerate(cols):
                            s_ap = get_s(cid)
                            nc.tensor.matmul(
                                out=psum_u[:, :],
                                lhsT=g_tiles[b][:, ch, :],
                                rhs=s_ap,
                                start=(j == 0), stop=(j == nlast),
                            )
                        u16 = xfer.tile([P, P], mybir.dt.float16, tag="u")
                        nc.scalar.activation(
                            out=u16, in_=psum_u,
                            func=mybir.ActivationFunctionType.Copy)
                        q = s - u0
                        nc.tensor.matmul(
                            out=psum_o[:, q * P:(q + 1) * P], lhsT=u16,
                            rhs=w_t, start=True, stop=True,
                            skip_group_check=True)
                    flush_pending()
                    state["pending"] = (psum_o, (st_g + u0) * P, uw * P, g)
            flush_pending()
            while state["wrote"] < NG:
                h = state["wrote"]
                st_h, nb_h = cfg.GROUPS[h]
                c0, c1 = st_h * P, (st_h + nb_h) * P
                nc.sync.dma_start(out=t_out[:, c0:c1], in_=out_t[:, c0:c1])
                state["wrote"] = h + 1
    nc.compile()
    return nc


def _unshard(cfg, meta, res):
    TPC, NSB = cfg.TPC, cfg.NSB
    outs = []
    for c in range(cfg.NCORES):
        o = res.results[c]["out"]                       # [P, NSB*P] fp16
        o = np.asarray(o, np.float32).reshape(P, NSB, P)
        o = o.transpose(1, 0, 2).reshape(NSB * P, P)[:TPC]
        outs.append(o)
    virt = np.concatenate(outs, axis=0)
    return virt[meta["perm"]].astype(np.float32)


def kernel(X, embs, W, edge_index, edge_weight):
    cfg = _REAL
    meta, in_maps = _host_prep(cfg, X, embs, W, edge_index, edge_weight)
    nc = _build_program(cfg, meta)
    res = run_bass_kernel_spmd(nc, in_maps, list(range(cfg.NCORES)))
    return _unshard(cfg, meta, res)


# revision 47
# speedup vs baseline: 3.1284x; 1.0338x over previous
"""GCNConv-style message passing kernel for Trainium2, 8 NeuronCores.

Reference semantics:
    deg  = 1 + segment_sum(edge_weight, col)           # self-loop included
    dinv = deg ** -0.5
    h    = embs @ W
    out[t] = (sum_e norm_e * h[src_e] + dinv[t]^2 * h[t]) * X[t],
             norm_e = dinv[src_e] * ew_e * dinv[t]

Device formulation (matmul commutes past the segment sum):
    embs8 = e3m4(SCALE * dinv[:, None] * embs)          (host, fp8 e3m4)
    u[t]  = sum_{e: col=t} ew_e * embs8[src_e]          (self loop folded in
                                                         as an extra edge)
    out[t] = (fp16(u[t]) @ W16) * fp16(dinv[t] * X[t] / SCALE)

Sharding: targets split across 8 cores (12500 each). Edges (incl. self
edges, whose sources point into a per-core virtual row region so the
layout is core-uniform) are grouped by (dest-block-group of SBG x 128
targets, source bank of 32768 rows) into slot segments; within a segment,
per-(dest-block, bank) slot spans are sized max-over-cores so the chunk
schedule is shared SPMD. Source rows are fetched with a raw 128-byte fp8
dma_gather (256B DRAM stride). Selection matrices S[e, t_loc] for BATCH
chunks at a time are built by ONE DVE tensor_tensor is_equal (2x fp16
mode) into a column-major [128, 128, BATCH] tile; chunk j's S is the
stride-BATCH slice [:, :, j], consumed by PE matmuls accumulating PSUM
u^T[cin, t_loc]. Chunks straddling dest-block boundaries emit one masked
S column per covered block. ACT copies PSUM->SBUF as fp16, PE applies W
into 4-block-wide PSUM tiles, DVE multiplies by the gating (deferred one
unit to avoid stalls) and the result is written back once as fp16.
"""

import numpy as np
import ml_dtypes

import concourse.bacc as bacc
import concourse.tile as tile
from concourse import mybir
from concourse.bass import exact_div
from concourse.bass_utils import run_bass_kernel_spmd

P = 128
BATCH = 16            # S columns built per DVE instruction
UNIT = 4              # dest blocks per output psum tile
E3M4 = ml_dtypes.float8_e3m4
SCALE = 4.0
E3M4_MAX = 15.5


class _Cfg:
    def __init__(self, n, n_cores, sb_group):
        self.N = n
        self.NCORES = n_cores
        self.TPC = n // n_cores              # targets per core
        assert self.TPC * n_cores == n
        self.NSB = -(-self.TPC // P)         # dest blocks of 128 per core
        self.BANK = 32768                    # gather bank rows (int16 idx)
        self.NV = n + self.TPC               # rows incl. per-core self region
        self.NBANK = -(-self.NV // self.BANK)
        # taper the last groups so the end-of-pipeline drain is short
        tail = [4, 3, 2, 1]
        sizes = []
        rem = self.NSB - sum(tail)
        if rem < sb_group:
            tail, rem = [], self.NSB
        while rem >= sb_group:
            sizes.append(sb_group)
            rem -= sb_group
        if rem:
            sizes.append(rem)
        sizes += tail
        assert sum(sizes) == self.NSB
        self.GROUPS = []                     # (start_block, nblocks)
        s0 = 0
        for sz in sizes:
            self.GROUPS.append((s0, sz))
            s0 += sz
        self.NG = len(self.GROUPS)
        self.SBGMAX = max(sz for _, sz in self.GROUPS)
        # block -> (group, local index)
        self.G_OF_SB = np.zeros(self.NSB, np.int64)
        self.SBL_OF_SB = np.zeros(self.NSB, np.int64)
        for g, (st, sz) in enumerate(self.GROUPS):
            self.G_OF_SB[st:st + sz] = g
            self.SBL_OF_SB[st:st + sz] = np.arange(sz)


_REAL = _Cfg(n=100000, n_cores=8, sb_group=8)


def _dma_gather_raw(gp, out_ap, in_ap, idxs_ap, num_idxs, elem_size,
                    elem_step, single_packet=False):
    """bass dma_gather clone (DRAM src, non-transpose) without the
    elem_size%256B restriction; elem_step sets the 256B-unit DRAM stride."""
    assert idxs_ap.dtype == mybir.dt.int16
    assert in_ap.dtype == out_ap.dtype
    assert in_ap.ap[-1][1] == elem_size
    assert out_ap.ap[-1][1] == elem_size
    assert out_ap.ap[0][1] * out_ap.ap[1][1] == ((num_idxs + 127) // 128) * 128
    assert in_ap.ap[0][0] == elem_step
    stride_bytes_256 = exact_div(elem_step * mybir.dt.size(in_ap.dtype), 256)
    assert 0 < stride_bytes_256 < 256
    _in_ap = gp.lower_ap_dma(in_ap, for_custom_bir_dma=True)
    _idxs_ap = gp.lower_ap(idxs_ap)
    _out_ap = gp.lower_ap(out_ap)
    return gp.add_instruction(
        mybir.InstDMAGatherAnt(
            name=gp.bass.get_next_instruction_name(),
            ins=[*_in_ap, _idxs_ap, gp.lower_val_access(gp.to_reg(num_idxs))],
            outs=[_out_ap],
            transpose=False,
            num_idxs=num_idxs,
            elem_size=elem_size,
            stride_bytes_256=stride_bytes_256,
            gen_mode=0,
            single_packet=single_packet,
            queue_num=0,
            sbuf_tokens_per_rank=0,
            sbuf_free_dim_per_rank=0,
            sbuf_free_dim_pad_per_rank=0,
            sbuf_byte_offset=0,
        )
    )


def _bcast_mid(ap, rep):
    """[P, k] AP -> [P, rep, k] with a stride-0 middle dim."""
    cls = type(ap)
    new = [list(ap.ap[0]), [0, rep], list(ap.ap[1])]
    return cls(ap.tensor, ap.offset, new)


def _balance_perm(cfg, src, col):
    """Greedy target -> virtual-id assignment equalizing per-(block
    position, source bank) in-degree across cores, which shrinks the
    max-over-cores slot padding. perm[t] = virtual id."""
    N, TPC, NSB, BANK, NBANK = cfg.N, cfg.TPC, cfg.NSB, cfg.BANK, cfg.NBANK
    NC = cfg.NCORES
    dkey = col * NBANK + src // BANK
    d = np.bincount(dkey, minlength=N * NBANK).reshape(N, NBANK)
    tot = d.sum(1)
    order = np.argsort(-tot, kind="stable")
    perm = np.empty(N, np.int64)
    pos = 0
    for k in range(NSB):
        blkN = min(P, TPC - k * P)
        cand = order[pos: pos + blkN * NC]
        pos += blkN * NC
        cnt = np.zeros(NC, np.int64)
        R = np.zeros((NC, NBANK), np.int64)
        for t in cand:
            # cost: increase of sum_b max_c R[c,b] (the actual padding),
            # tie-broken toward the least-loaded core
            curmax = R.max(axis=0)
            inc = np.maximum(R + d[t] - curmax, 0).sum(axis=1)
            inc = inc.astype(np.float64) + 1e-3 * R.sum(axis=1)
            inc[cnt >= blkN] = 1e18
            c = int(np.argmin(inc))
            perm[t] = c * TPC + k * P + cnt[c]
            R[c] += d[t]
            cnt[c] += 1
    return perm


def _host_prep(cfg, X, embs, W, edge_index, edge_weight):
    """Build fp8 embs table, slot layout, chunk schedule, per-core arrays."""
    N, TPC, NSB, BANK, NBANK = cfg.N, cfg.TPC, cfg.NSB, cfg.BANK, cfg.NBANK
    NC, SBG, NG = cfg.NCORES, cfg.SBGMAX, cfg.NG

    src = np.asarray(edge_index[0], dtype=np.int64)
    col = np.asarray(edge_index[1], dtype=np.int64)
    ew = np.asarray(edge_weight, dtype=np.float32)
    ew_ones = bool(np.all(ew == 1.0))

    perm = _balance_perm(cfg, src, col)
    inv = np.empty(N, np.int64)
    inv[perm] = np.arange(N)

    # self loops as ordinary edges; their source points into a per-core
    # virtual row region [N, N+TPC) so the (block, bank) slot layout is
    # identical across cores. Targets are remapped through perm.
    loop = np.arange(N, dtype=np.int64)
    src_a = np.concatenate([src, N + (perm[loop] % TPC)])
    col_a = np.concatenate([perm[col], perm[loop]])
    ew_a = np.concatenate([ew, np.ones(N, np.float32)])

    deg = 1.0 + np.bincount(col, weights=ew.astype(np.float64), minlength=N)
    dinv = (1.0 / np.sqrt(deg)).astype(np.float32)

    embs_s = dinv[:, None] * np.asarray(embs, np.float32)
    embs8 = np.zeros((cfg.NV, 256), E3M4)
    embs8[:N, :P] = np.clip(embs_s * SCALE, -E3M4_MAX, E3M4_MAX).astype(E3M4)
    gx = (dinv[:, None] * np.asarray(X, np.float32)) * (1.0 / SCALE)

    # bucket keys
    core = col_a // TPC
    sb = (col_a % TPC) // P                  # 0..NSB-1
    grp = cfg.G_OF_SB[sb]
    sbl = cfg.SBL_OF_SB[sb]                  # block local to group
    bank = src_a // BANK

    key = ((core * NG + grp) * NBANK + bank) * SBG + sbl
    nkey = NC * NG * NBANK * SBG
    counts = np.bincount(key, minlength=nkey).reshape(NC, NG, NBANK, SBG)
    M = counts.max(axis=0)                   # shared span sizes [NG,NBANK,SBG]

    # shared slot layout: segments (g,b) in order, blocks in order inside,
    # each segment padded to a 128 multiple.
    blk_off = np.zeros((NG, NBANK, SBG), np.int64)   # block span start
    seg_base = np.zeros((NG, NBANK), np.int64)       # segment slot base
    nch = np.zeros((NG, NBANK), np.int64)            # chunks per segment
    pos = 0
    for g in range(NG):
        nb = cfg.GROUPS[g][1]
        for b in range(NBANK):
            seg_base[g, b] = pos
            off = 0
            for s in range(nb):
                blk_off[g, b, s] = off
                off += M[g, b, s]
            nch[g, b] = -(-off // P)
            pos += nch[g, b] * P
    slots_tot = pos

    # chunk/block overlap -> tloc columns, in device processing order
    colid = {}
    sched = []                               # [g][s_local] -> [(b, ch, col)]
    ncols = 0
    for g in range(NG):
        nb = cfg.GROUPS[g][1]
        gsched = []
        for s in range(nb):
            lst = []
            for b in range(NBANK):
                lo = blk_off[g, b, s]
                hi = lo + M[g, b, s]
                if hi == lo:
                    continue
                c0, c1 = lo // P, (hi - 1) // P
                for c in range(c0, c1 + 1):
                    colid[(g, b, c, s)] = ncols
                    lst.append((b, int(c), ncols))
                    ncols += 1
            assert lst, f"block {g},{s} has no slots"
            gsched.append(lst)
        sched.append(gsched)
    ncols_pad = -(-ncols // BATCH) * BATCH

    # per-edge destination slots
    cnt_flat = counts.reshape(-1)
    order = np.argsort(key, kind="stable")
    starts = np.zeros_like(cnt_flat)
    np.cumsum(cnt_flat[:-1], out=starts[1:])
    k_sorted = key[order]
    rank = np.arange(len(order)) - starts[k_sorted]
    g_s, b_s = grp[order], bank[order]
    sbl_s, core_s = sbl[order], core[order]
    dest = (seg_base[g_s, b_s] + blk_off[g_s, b_s, sbl_s] + rank)
    chunk_s = (dest - seg_base[g_s, b_s]) // P    # segment-local chunk
    lane_s = dest % P
    tl_s = (col_a[order] % TPC % P).astype(np.float32)
    src_l = (src_a[order] - b_s * BANK).astype(np.int16)
    ew_s = ew_a[order]

    # column index per edge (vectorized via dict -> array)
    mchunk = slots_tot // P + 1
    ckey = ((g_s * NBANK + b_s) * mchunk + chunk_s) * SBG + sbl_s
    uk, uinv = np.unique(ckey, return_inverse=True)
    uk_col = np.empty(len(uk), np.int64)
    for i, k in enumerate(uk):
        sblk = k % SBG
        k //= SBG
        ch = k % mchunk
        k //= mchunk
        b = k % NBANK
        g = k // NBANK
        uk_col[i] = colid[(g, b, ch, sblk)]
    col_e = uk_col[uinv]

    IDX = np.zeros((NC, slots_tot), np.int16)
    TLOC = np.full((NC, P, ncols_pad), -100.0, np.float16)
    IDX[core_s, dest] = src_l
    TLOC[core_s, lane_s, col_e] = tl_s
    EWC = None
    if not ew_ones:
        EWC = np.ones((NC, P, ncols_pad), np.float16)
        EWC[core_s, lane_s, col_e] = ew_s

    # pack gather indices wrap-16, replicate to 128 partitions
    idx_packed = IDX.reshape(NC, slots_tot // 16, 16).transpose(0, 2, 1)
    idx_all = np.tile(idx_packed, (1, 8, 1)).astype(np.int16)

    # partition-major gx layout in virtual row order
    npad = NSB * P
    gx_v = gx[inv]
    gx_pm = np.zeros((NC, P, npad), np.float16)
    for c in range(NC):
        gxc = gx_v[c * TPC:(c + 1) * TPC]
        gxc = np.concatenate(
            [gxc, np.zeros((npad - TPC, P), np.float32)], axis=0)
        gx_pm[c] = gxc.reshape(NSB, P, P).transpose(1, 0, 2).reshape(
            P, npad).astype(np.float16)

    # iotacm[p, t*BATCH + j] = t  (column-major S layout constant)
    iotacm = np.repeat(np.arange(P, dtype=np.float16), BATCH)[None, :]
    iotacm = np.tile(iotacm, (P, 1))
    w16 = np.asarray(W, np.float16)

    meta = dict(sched=sched, nch=nch, seg_base=seg_base, slots_tot=slots_tot,
                ncols=ncols, ncols_pad=ncols_pad, ew_ones=ew_ones, perm=perm)
    in_maps = []
    for c in range(NC):
        e8 = embs8.copy()
        e8[N:N + TPC] = embs8[inv[c * TPC:(c + 1) * TPC]]
        m = dict(
            embs8=e8,
            w16=w16,
            gx=np.ascontiguousarray(gx_pm[c]),
            idxall=np.ascontiguousarray(idx_all[c]),
            tlocall=np.ascontiguousarray(TLOC[c].reshape(P, ncols_pad)),
            iotacm=iotacm,
        )
        if EWC is not None:
            m["ewall"] = np.ascontiguousarray(EWC[c].reshape(P, ncols_pad))
        in_maps.append(m)
    return meta, in_maps


def _build_program(cfg, meta):
    N, TPC, NSB, BANK, NBANK = cfg.N, cfg.TPC, cfg.NSB, cfg.BANK, cfg.NBANK
    NG = cfg.NG
    sched, nch, seg_base = meta["sched"], meta["nch"], meta["seg_base"]
    slots_tot, ncols_pad, ew_ones = (meta["slots_tot"], meta["ncols_pad"],
                                     meta["ew_ones"])
    npad = NSB * P

    nc = bacc.Bacc("TRN2", target_bir_lowering=False, debug=False,
                   num_devices=cfg.NCORES)
    t_embs8 = nc.dram_tensor("embs8", [cfg.NV, 256], mybir.dt.float8e3,
                             kind="ExternalInput").ap()
    t_w = nc.dram_tensor("w16", [P, P], mybir.dt.float16,
                         kind="ExternalInput").ap()
    t_gx = nc.dram_tensor("gx", [P, npad], mybir.dt.float16,
                          kind="ExternalInput").ap()
    t_idx = nc.dram_tensor("idxall", [P, slots_tot // 16], mybir.dt.int16,
                           kind="ExternalInput").ap()
    t_tloc = nc.dram_tensor("tlocall", [P, ncols_pad], mybir.dt.float16,
                            kind="ExternalInput").ap()
    t_iotacm = nc.dram_tensor("iotacm", [P, P * BATCH], mybir.dt.float16,
                              kind="ExternalInput").ap()
    t_ew = None
    if not ew_ones:
        t_ew = nc.dram_tensor("ewall", [P, ncols_pad], mybir.dt.float16,
                              kind="ExternalInput").ap()
    t_out = nc.dram_tensor("out", [P, npad], mybir.dt.float16,
                           kind="ExternalOutput").ap()

    with tile.TileContext(nc) as tc:
        with tc.tile_pool(name="const", bufs=1) as cpool, \
             tc.tile_pool(name="gpool", bufs=3) as gpool, \
             tc.tile_pool(name="spool", bufs=6) as spool, \
             tc.tile_pool(name="xfer", bufs=4) as xfer, \
             tc.tile_pool(name="psu", bufs=4, space="PSUM") as psu, \
             tc.tile_pool(name="psb", bufs=3, space="PSUM") as psb:

            tloc_t = cpool.tile([P, ncols_pad], mybir.dt.float16)
            nc.sync.dma_start(out=tloc_t, in_=t_tloc)
            iotacm_t = cpool.tile([P, P, BATCH], mybir.dt.float16)
            nc.sync.dma_start(out=iotacm_t, in_=t_iotacm)
            w_t = cpool.tile([P, P], mybir.dt.float16)
            nc.sync.dma_start(out=w_t, in_=t_w)
            idx_t = cpool.tile([P, slots_tot // 16], mybir.dt.int16)
            ew_t = None
            if t_ew is not None:
                ew_t = cpool.tile([P, ncols_pad], mybir.dt.float16)
                nc.sync.dma_start(out=ew_t, in_=t_ew)
            gx_t = cpool.tile([P, npad], mybir.dt.float16)
            out_t = cpool.tile([P, npad], mybir.dt.float16)
            gx_loaded = False

            state = dict(batch=-1, s_cm=None, pending=None, wrote=0)

            def get_s(cid):
                bi = cid // BATCH
                if bi != state["batch"]:
                    s_cm = spool.tile([P, P, BATCH], mybir.dt.float16,
                                      tag="s")
                    nc.vector.tensor_tensor(
                        out=s_cm[:, :, :], in0=iotacm_t[:, :, :],
                        in1=_bcast_mid(tloc_t[:, bi * BATCH:(bi + 1) * BATCH],
                                       P),
                        op=mybir.AluOpType.is_equal,
                    )
                    if ew_t is not None:
                        s2 = spool.tile([P, P, BATCH], mybir.dt.float16,
                                        tag="s2")
                        nc.vector.tensor_tensor(
                            out=s2[:, :, :], in0=s_cm[:, :, :],
                            in1=_bcast_mid(
                                ew_t[:, bi * BATCH:(bi + 1) * BATCH], P),
                            op=mybir.AluOpType.mult,
                        )
                        s_cm = s2
                    state["batch"] = bi
                    state["s_cm"] = s_cm
                return state["s_cm"][:, :, cid % BATCH]

            def flush_pending():
                if state["pending"] is None:
                    return
                po, sg, w, gtag = state["pending"]
                nc.vector.tensor_tensor(
                    out=out_t[:, sg:sg + w], in0=po[:, :w],
                    in1=gx_t[:, sg:sg + w], op=mybir.AluOpType.mult)
                state["pending"] = None
                while state["wrote"] < gtag:     # groups < gtag are complete
                    h = state["wrote"]
                    st_h, nb_h = cfg.GROUPS[h]
                    c0, c1 = st_h * P, (st_h + nb_h) * P
                    nc.sync.dma_start(out=t_out[:, c0:c1],
                                      in_=out_t[:, c0:c1])
                    state["wrote"] = h + 1

            for g in range(NG):
                st_g, nb = cfg.GROUPS[g]
                g_tiles = []
                for b in range(NBANK):
                    nseg = int(nch[g, b])
                    if nseg == 0:
                        g_tiles.append(None)
                        continue
                    g_t = gpool.tile([P, nseg, P], mybir.dt.float8e3,
                                     tag=f"g{b}")
                    rows = min(BANK, cfg.NV - b * BANK)
                    off = int(seg_base[g, b]) // 16
                    nc.sync.dma_start(out=idx_t[:, off:off + nseg * 8],
                                      in_=t_idx[:, off:off + nseg * 8])
                    _dma_gather_raw(
                        nc.gpsimd, g_t[:, :, :],
                        t_embs8[b * BANK: b * BANK + rows, 0:P],
                        idx_t[:, off:off + nseg * 8],
                        num_idxs=nseg * P, elem_size=P, elem_step=256,
                    )
                    g_tiles.append(g_t)
                if not gx_loaded:
                    # deferred so the first gathers go out first
                    nc.sync.dma_start(out=gx_t, in_=t_gx)
                    gx_loaded = True

                for u0 in range(0, nb, UNIT):
                    uw = min(UNIT, nb - u0)
                    psum_o = psb.tile([P, UNIT * P], mybir.dt.float32,
                                      space="PSUM")
                    for s in range(u0, u0 + uw):
                        cols = sched[g][s]
                        psum_u = psu.tile([P, P], mybir.dt.float32,
                                          space="PSUM")
                        nlast = len(cols) - 1
                        for j, (b, ch, cid) in enumerate(cols):
                            s_ap = get_s(cid)
                            nc.tensor.matmul(
                                out=psum_u[:, :],
                                lhsT=g_tiles[b][:, ch, :],
                                rhs=s_ap,
                                start=(j == 0), stop=(j == nlast),
                            )
                        u16 = xfer.tile([P, P], mybir.dt.float16, tag="u")
                        nc.scalar.activation(
                            out=u16, in_=psum_u,
                            func=mybir.ActivationFunctionType.Copy)
                        q = s - u0
                        nc.tensor.matmul(
                            out=psum_o[:, q * P:(q + 1) * P], lhsT=u16,
                            rhs=w_t, start=True, stop=True,
                            skip_group_check=True)
                    flush_pending()
                    state["pending"] = (psum_o, (st_g + u0) * P, uw * P, g)
            flush_pending()
            while state["wrote"] < NG:
                h = state["wrote"]
                st_h, nb_h = cfg.GROUPS[h]
                c0, c1 = st_h * P, (st_h + nb_h) * P
                nc.sync.dma_start(out=t_out[:, c0:c1], in_=out_t[:, c0:c1])
                state["wrote"] = h + 1
    nc.compile()
    return nc


def _unshard(cfg, meta, res):
    TPC, NSB = cfg.TPC, cfg.NSB
    outs = []
    for c in range(cfg.NCORES):
        o = res.results[c]["out"]                       # [P, NSB*P] fp16
        o = np.asarray(o, np.float32).reshape(P, NSB, P)
        o = o.transpose(1, 0, 2).reshape(NSB * P, P)[:TPC]
        outs.append(o)
    virt = np.concatenate(outs, axis=0)
    return virt[meta["perm"]].astype(np.float32)


def kernel(X, embs, W, edge_index, edge_weight):
    cfg = _REAL
    meta, in_maps = _host_prep(cfg, X, embs, W, edge_index, edge_weight)
    nc = _build_program(cfg, meta)
    res = run_bass_kernel_spmd(nc, in_maps, list(range(cfg.NCORES)))
    return _unshard(cfg, meta, res)


# revision 63
# speedup vs baseline: 3.1788x; 1.0161x over previous
"""GCNConv-style message passing kernel for Trainium2, 8 NeuronCores.

Reference semantics:
    deg  = 1 + segment_sum(edge_weight, col)           # self-loop included
    dinv = deg ** -0.5
    h    = embs @ W
    out[t] = (sum_e norm_e * h[src_e] + dinv[t]^2 * h[t]) * X[t],
             norm_e = dinv[src_e] * ew_e * dinv[t]

Device formulation (matmul commutes past the segment sum):
    embs8 = e3m4(SCALE * dinv[:, None] * embs)          (host, fp8 e3m4)
    u[t]  = sum_{e: col=t} ew_e * embs8[src_e]          (self loop folded in
                                                         as an extra edge)
    out[t] = (fp16(u[t]) @ W16) * fp16(dinv[t] * X[t] / SCALE)

Sharding: targets split across 8 cores (12500 each). Edges (incl. self
edges, whose sources point into a per-core virtual row region so the
layout is core-uniform) are grouped by (dest-block-group of SBG x 128
targets, source bank of 32768 rows) into slot segments; within a segment,
per-(dest-block, bank) slot spans are sized max-over-cores so the chunk
schedule is shared SPMD. Source rows are fetched with a raw 128-byte fp8
dma_gather (256B DRAM stride). Selection matrices S[e, t_loc] for BATCH
chunks at a time are built by ONE DVE tensor_tensor is_equal (2x fp16
mode) into a column-major [128, 128, BATCH] tile; chunk j's S is the
stride-BATCH slice [:, :, j], consumed by PE matmuls accumulating PSUM
u^T[cin, t_loc]. Chunks straddling dest-block boundaries emit one masked
S column per covered block. ACT copies PSUM->SBUF as fp16, PE applies W
into 4-block-wide PSUM tiles, DVE multiplies by the gating (deferred one
unit to avoid stalls) and the result is written back once as fp16.
"""

import numpy as np
import ml_dtypes

import concourse.bacc as bacc
import concourse.tile as tile
from concourse import mybir
from concourse.bass import exact_div
from concourse.bass_utils import run_bass_kernel_spmd

P = 128
BATCH = 16            # S columns built per DVE instruction
UNIT = 4              # dest blocks per output psum tile
E3M4 = ml_dtypes.float8_e3m4
SCALE = 4.0
E3M4_MAX = 15.5


class _Cfg:
    def __init__(self, n, n_cores, sb_group):
        self.N = n
        self.NCORES = n_cores
        self.TPC = n // n_cores              # targets per core
        assert self.TPC * n_cores == n
        self.NSB = -(-self.TPC // P)         # dest blocks of 128 per core
        self.BANK = 32768                    # gather bank rows (int16 idx)
        self.NV = n + self.TPC               # rows incl. per-core self region
        self.NBANK = -(-self.NV // self.BANK)
        # taper the last groups so the end-of-pipeline drain is short
        tail = [4, 3, 2, 1]
        sizes = []
        rem = self.NSB - sum(tail)
        if rem < sb_group:
            tail, rem = [], self.NSB
        while rem >= sb_group:
            sizes.append(sb_group)
            rem -= sb_group
        if rem:
            sizes.append(rem)
        sizes += tail
        assert sum(sizes) == self.NSB
        self.GROUPS = []                     # (start_block, nblocks)
        s0 = 0
        for sz in sizes:
            self.GROUPS.append((s0, sz))
            s0 += sz
        self.NG = len(self.GROUPS)
        self.SBGMAX = max(sz for _, sz in self.GROUPS)
        # block -> (group, local index)
        self.G_OF_SB = np.zeros(self.NSB, np.int64)
        self.SBL_OF_SB = np.zeros(self.NSB, np.int64)
        for g, (st, sz) in enumerate(self.GROUPS):
            self.G_OF_SB[st:st + sz] = g
            self.SBL_OF_SB[st:st + sz] = np.arange(sz)


_REAL = _Cfg(n=100000, n_cores=8, sb_group=8)


def _dma_gather_raw(gp, out_ap, in_ap, idxs_ap, num_idxs, elem_size,
                    elem_step, single_packet=False):
    """bass dma_gather clone (DRAM src, non-transpose) without the
    elem_size%256B restriction; elem_step sets the 256B-unit DRAM stride."""
    assert idxs_ap.dtype == mybir.dt.int16
    assert in_ap.dtype == out_ap.dtype
    assert in_ap.ap[-1][1] == elem_size
    assert out_ap.ap[-1][1] == elem_size
    assert out_ap.ap[0][1] * out_ap.ap[1][1] == ((num_idxs + 127) // 128) * 128
    assert in_ap.ap[0][0] == elem_step
    stride_bytes_256 = exact_div(elem_step * mybir.dt.size(in_ap.dtype), 256)
    assert 0 < stride_bytes_256 < 256
    _in_ap = gp.lower_ap_dma(in_ap, for_custom_bir_dma=True)
    _idxs_ap = gp.lower_ap(idxs_ap)
    _out_ap = gp.lower_ap(out_ap)
    return gp.add_instruction(
        mybir.InstDMAGatherAnt(
            name=gp.bass.get_next_instruction_name(),
            ins=[*_in_ap, _idxs_ap, gp.lower_val_access(gp.to_reg(num_idxs))],
            outs=[_out_ap],
            transpose=False,
            num_idxs=num_idxs,
            elem_size=elem_size,
            stride_bytes_256=stride_bytes_256,
            gen_mode=0,
            single_packet=single_packet,
            queue_num=0,
            sbuf_tokens_per_rank=0,
            sbuf_free_dim_per_rank=0,
            sbuf_free_dim_pad_per_rank=0,
            sbuf_byte_offset=0,
        )
    )


def _bcast_mid(ap, rep):
    """[P, k] AP -> [P, rep, k] with a stride-0 middle dim."""
    cls = type(ap)
    new = [list(ap.ap[0]), [0, rep], list(ap.ap[1])]
    return cls(ap.tensor, ap.offset, new)


def _as3d(ap, mid, last):
    """[P, mid*last] contiguous AP -> [P, mid, last] view."""
    cls = type(ap)
    assert list(ap.ap[1]) == [1, mid * last]
    new = [list(ap.ap[0]), [last, mid], [1, last]]
    return cls(ap.tensor, ap.offset, new)


def _balance_perm(cfg, src, col):
    """Greedy target -> virtual-id assignment equalizing per-(block
    position, source bank) in-degree across cores, which shrinks the
    max-over-cores slot padding. perm[t] = virtual id."""
    N, TPC, NSB, BANK, NBANK = cfg.N, cfg.TPC, cfg.NSB, cfg.BANK, cfg.NBANK
    NC = cfg.NCORES
    dkey = col * NBANK + src // BANK
    d = np.bincount(dkey, minlength=N * NBANK).reshape(N, NBANK)
    tot = d.sum(1)
    order = np.argsort(-tot, kind="stable")
    perm = np.empty(N, np.int64)
    pos = 0
    for k in range(NSB):
        blkN = min(P, TPC - k * P)
        cand = order[pos: pos + blkN * NC]
        pos += blkN * NC
        cnt = np.zeros(NC, np.int64)
        R = np.zeros((NC, NBANK), np.int64)
        for t in cand:
            # cost: increase of sum_b max_c R[c,b] (the actual padding),
            # tie-broken toward the least-loaded core
            curmax = R.max(axis=0)
            inc = np.maximum(R + d[t] - curmax, 0).sum(axis=1)
            inc = inc.astype(np.float64) + 1e-3 * R.sum(axis=1)
            inc[cnt >= blkN] = 1e18
            c = int(np.argmin(inc))
            perm[t] = c * TPC + k * P + cnt[c]
            R[c] += d[t]
            cnt[c] += 1
    return perm


def _host_prep(cfg, X, embs, W, edge_index, edge_weight):
    """Build fp8 embs table, slot layout, chunk schedule, per-core arrays."""
    N, TPC, NSB, BANK, NBANK = cfg.N, cfg.TPC, cfg.NSB, cfg.BANK, cfg.NBANK
    NC, SBG, NG = cfg.NCORES, cfg.SBGMAX, cfg.NG

    src = np.asarray(edge_index[0], dtype=np.int64)
    col = np.asarray(edge_index[1], dtype=np.int64)
    ew = np.asarray(edge_weight, dtype=np.float32)
    ew_ones = bool(np.all(ew == 1.0))

    perm = _balance_perm(cfg, src, col)
    inv = np.empty(N, np.int64)
    inv[perm] = np.arange(N)

    # self loops as ordinary edges; their source points into a per-core
    # virtual row region [N, N+TPC) so the (block, bank) slot layout is
    # identical across cores. Targets are remapped through perm.
    loop = np.arange(N, dtype=np.int64)
    src_a = np.concatenate([src, N + (perm[loop] % TPC)])
    col_a = np.concatenate([perm[col], perm[loop]])
    ew_a = np.concatenate([ew, np.ones(N, np.float32)])

    deg = 1.0 + np.bincount(col, weights=ew.astype(np.float64), minlength=N)
    dinv = (1.0 / np.sqrt(deg)).astype(np.float32)

    embs_s = dinv[:, None] * np.asarray(embs, np.float32)
    embs8 = np.zeros((cfg.NV, 256), E3M4)
    embs8[:N, :P] = np.clip(embs_s * SCALE, -E3M4_MAX, E3M4_MAX).astype(E3M4)
    gx = (dinv[:, None] * np.asarray(X, np.float32)) * (1.0 / SCALE)

    # bucket keys
    core = col_a // TPC
    sb = (col_a % TPC) // P                  # 0..NSB-1
    grp = cfg.G_OF_SB[sb]
    sbl = cfg.SBL_OF_SB[sb]                  # block local to group
    bank = src_a // BANK

    key = ((core * NG + grp) * NBANK + bank) * SBG + sbl
    nkey = NC * NG * NBANK * SBG
    counts = np.bincount(key, minlength=nkey).reshape(NC, NG, NBANK, SBG)
    M = counts.max(axis=0)                   # shared span sizes [NG,NBANK,SBG]

    # shared slot layout: segments (g,b) in order, blocks in order inside,
    # each segment padded to a 128 multiple.
    blk_off = np.zeros((NG, NBANK, SBG), np.int64)   # block span start
    seg_base = np.zeros((NG, NBANK), np.int64)       # segment slot base
    nch = np.zeros((NG, NBANK), np.int64)            # chunks per segment
    seg_len = np.zeros((NG, NBANK), np.int64)        # real slots (16-aligned)
    pos = 0
    for g in range(NG):
        nb = cfg.GROUPS[g][1]
        for b in range(NBANK):
            seg_base[g, b] = pos
            off = 0
            for s in range(nb):
                blk_off[g, b, s] = off
                off += M[g, b, s]
            nch[g, b] = -(-off // P)
            seg_len[g, b] = -(-off // 16) * 16
            pos += nch[g, b] * P
    slots_tot = pos

    # chunk/block overlap -> tloc columns, in device processing order
    colid = {}
    sched = []                               # [g][s_local] -> [(b, ch, col)]
    ncols = 0
    for g in range(NG):
        nb = cfg.GROUPS[g][1]
        gsched = []
        for s in range(nb):
            lst = []
            for b in range(NBANK):
                lo = blk_off[g, b, s]
                hi = lo + M[g, b, s]
                if hi == lo:
                    continue
                c0, c1 = lo // P, (hi - 1) // P
                for c in range(c0, c1 + 1):
                    colid[(g, b, c, s)] = ncols
                    lst.append((b, int(c), ncols))
                    ncols += 1
            assert lst, f"block {g},{s} has no slots"
            gsched.append(lst)
        sched.append(gsched)
    ncols_pad = -(-ncols // BATCH) * BATCH

    # per-edge destination slots
    cnt_flat = counts.reshape(-1)
    order = np.argsort(key, kind="stable")
    starts = np.zeros_like(cnt_flat)
    np.cumsum(cnt_flat[:-1], out=starts[1:])
    k_sorted = key[order]
    rank = np.arange(len(order)) - starts[k_sorted]
    g_s, b_s = grp[order], bank[order]
    sbl_s, core_s = sbl[order], core[order]
    dest = (seg_base[g_s, b_s] + blk_off[g_s, b_s, sbl_s] + rank)
    chunk_s = (dest - seg_base[g_s, b_s]) // P    # segment-local chunk
    lane_s = dest % P
    tl_s = (col_a[order] % TPC % P).astype(np.float32)
    src_l = (src_a[order] - b_s * BANK).astype(np.int16)
    ew_s = ew_a[order]

    # column index per edge (vectorized via dict -> array)
    mchunk = slots_tot // P + 1
    ckey = ((g_s * NBANK + b_s) * mchunk + chunk_s) * SBG + sbl_s
    uk, uinv = np.unique(ckey, return_inverse=True)
    uk_col = np.empty(len(uk), np.int64)
    for i, k in enumerate(uk):
        sblk = k % SBG
        k //= SBG
        ch = k % mchunk
        k //= mchunk
        b = k % NBANK
        g = k // NBANK
        uk_col[i] = colid[(g, b, ch, sblk)]
    col_e = uk_col[uinv]

    IDX = np.zeros((NC, slots_tot), np.int16)
    TLOC = np.full((NC, P, ncols_pad), -100.0, np.float16)
    IDX[core_s, dest] = src_l
    TLOC[core_s, lane_s, col_e] = tl_s
    EWC = None
    if not ew_ones:
        EWC = np.ones((NC, P, ncols_pad), np.float16)
        EWC[core_s, lane_s, col_e] = ew_s

    # pack gather indices wrap-16; the Q7 gather ucode only reads idx
    # partitions 0-31 (cores 0-1 generate all descriptors), so replicate to
    # 32 partitions only; the device zero-fills partitions 32-127.
    idx_packed = IDX.reshape(NC, slots_tot // 16, 16).transpose(0, 2, 1)
    idx_all = np.tile(idx_packed, (1, 2, 1)).astype(np.int16)

    # partition-major gx layout in virtual row order
    npad = NSB * P
    gx_v = gx[inv]
    gx_pm = np.zeros((NC, P, npad), np.float16)
    for c in range(NC):
        gxc = gx_v[c * TPC:(c + 1) * TPC]
        gxc = np.concatenate(
            [gxc, np.zeros((npad - TPC, P), np.float32)], axis=0)
        gx_pm[c] = gxc.reshape(NSB, P, P).transpose(1, 0, 2).reshape(
            P, npad).astype(np.float16)

    # iotacm[p, t*BATCH + j] = t  (column-major S layout constant)
    iotacm = np.repeat(np.arange(P, dtype=np.float16), BATCH)[None, :]
    iotacm = np.tile(iotacm, (P, 1))
    w16 = np.asarray(W, np.float16)

    meta = dict(sched=sched, nch=nch, seg_base=seg_base, slots_tot=slots_tot,
                ncols=ncols, ncols_pad=ncols_pad, ew_ones=ew_ones, perm=perm)
    in_maps = []
    for c in range(NC):
        e8 = embs8.copy()
        e8[N:N + TPC] = embs8[inv[c * TPC:(c + 1) * TPC]]
        m = dict(
            embs8=e8,
            w16=w16,
            gx=np.ascontiguousarray(gx_pm[c]),
            idxall=np.ascontiguousarray(idx_all[c]),
            tlocall=np.ascontiguousarray(TLOC[c].reshape(P, ncols_pad)),
            iotacm=iotacm,
        )
        if EWC is not None:
            m["ewall"] = np.ascontiguousarray(EWC[c].reshape(P, ncols_pad))
        in_maps.append(m)
    return meta, in_maps


def _build_program(cfg, meta):
    N, TPC, NSB, BANK, NBANK = cfg.N, cfg.TPC, cfg.NSB, cfg.BANK, cfg.NBANK
    NG = cfg.NG
    sched, nch, seg_base = meta["sched"], meta["nch"], meta["seg_base"]
    slots_tot, ncols_pad, ew_ones = (meta["slots_tot"], meta["ncols_pad"],
                                     meta["ew_ones"])
    npad = NSB * P

    nc = bacc.Bacc("TRN2", target_bir_lowering=False, debug=False,
                   num_devices=cfg.NCORES)
    t_embs8 = nc.dram_tensor("embs8", [cfg.NV, 256], mybir.dt.float8e3,
                             kind="ExternalInput").ap()
    t_w = nc.dram_tensor("w16", [P, P], mybir.dt.float16,
                         kind="ExternalInput").ap()
    t_gx = nc.dram_tensor("gx", [P, npad], mybir.dt.float16,
                          kind="ExternalInput").ap()
    t_idx = nc.dram_tensor("idxall", [32, slots_tot // 16], mybir.dt.int16,
                           kind="ExternalInput").ap()
    t_tloc = nc.dram_tensor("tlocall", [P, ncols_pad], mybir.dt.float16,
                            kind="ExternalInput").ap()
    t_iotacm = nc.dram_tensor("iotacm", [P, P * BATCH], mybir.dt.float16,
                              kind="ExternalInput").ap()
    t_ew = None
    if not ew_ones:
        t_ew = nc.dram_tensor("ewall", [P, ncols_pad], mybir.dt.float16,
                              kind="ExternalInput").ap()
    t_out = nc.dram_tensor("out", [P, npad], mybir.dt.float16,
                           kind="ExternalOutput").ap()

    with tile.TileContext(nc) as tc:
        with tc.tile_pool(name="const", bufs=1) as cpool, \
             tc.tile_pool(name="gpool", bufs=3) as gpool, \
             tc.tile_pool(name="spool", bufs=6) as spool, \
             tc.tile_pool(name="xfer", bufs=4) as xfer, \
             tc.tile_pool(name="psu", bufs=4, space="PSUM") as psu, \
             tc.tile_pool(name="psb", bufs=3, space="PSUM") as psb:

            tloc_t = cpool.tile([P, ncols_pad], mybir.dt.float16)
            nc.sync.dma_start(out=tloc_t, in_=t_tloc)
            iotacm_t = cpool.tile([P, P * BATCH], mybir.dt.float16)
            nc.sync.dma_start(out=iotacm_t, in_=t_iotacm)
            w_t = cpool.tile([P, P], mybir.dt.float16)
            nc.sync.dma_start(out=w_t, in_=t_w)
            idx_t = cpool.tile([P, slots_tot // 16], mybir.dt.int16)
            ew_t = None
            if t_ew is not None:
                ew_t = cpool.tile([P, ncols_pad], mybir.dt.float16)
                nc.sync.dma_start(out=ew_t, in_=t_ew)
            gx_t = cpool.tile([P, npad], mybir.dt.float16)
            out_t = cpool.tile([P, npad], mybir.dt.float16)
            gx_loaded = False

            state = dict(batch=-1, s_cm=None, pending=None, wrote=0, wcol=0)

            def get_s(cid):
                bi = cid // BATCH
                if bi != state["batch"]:
                    s_cm = spool.tile([P, P, BATCH], mybir.dt.float16,
                                      tag="s")
                    nc.vector.tensor_tensor(
                        out=s_cm[:, :, :],
                        in0=_as3d(iotacm_t[:, :], P, BATCH),
                        in1=_bcast_mid(tloc_t[:, bi * BATCH:(bi + 1) * BATCH],
                                       P),
                        op=mybir.AluOpType.is_equal,
                    )
                    if ew_t is not None:
                        s2 = spool.tile([P, P, BATCH], mybir.dt.float16,
                                        tag="s2")
                        nc.vector.tensor_tensor(
                            out=s2[:, :, :], in0=s_cm[:, :, :],
                            in1=_bcast_mid(
                                ew_t[:, bi * BATCH:(bi + 1) * BATCH], P),
                            op=mybir.AluOpType.mult,
                        )
                        s_cm = s2
                    state["batch"] = bi
                    state["s_cm"] = s_cm
                return state["s_cm"][:, :, cid % BATCH]

            def flush_pending():
                if state["pending"] is None:
                    return
                po, sg, w, gtag = state["pending"]
                nc.vector.tensor_tensor(
                    out=out_t[:, sg:sg + w], in0=po[:, :w],
                    in1=gx_t[:, sg:sg + w], op=mybir.AluOpType.mult)
                state["pending"] = None
                if state["wrote"] < gtag:        # groups < gtag are complete
                    st_h, nb_h = cfg.GROUPS[gtag - 1]
                    c1 = (st_h + nb_h) * P
                    nc.sync.dma_start(out=t_out[:, state["wcol"]:c1],
                                      in_=out_t[:, state["wcol"]:c1])
                    state["wcol"] = c1
                    state["wrote"] = gtag

            for g in range(NG):
                st_g, nb = cfg.GROUPS[g]
                # zero partitions 32-127 of this group's idx column span (the
                # Q7 gather ucode reads only partitions 0-31; the rest just
                # needs defined, in-range values). Done on the idle ACT
                # engine: Copy with scale=0 of any resident fp16 tile.
                gc0 = int(seg_base[g, 0]) // 16
                gc1 = int(seg_base[g, NBANK - 1] + nch[g, NBANK - 1] * P) // 16
                if gc1 > gc0:
                    # ops with base partition > 0 are limited to 32 partitions
                    for p0 in (32, 64, 96):
                        nc.gpsimd.memset(idx_t[p0:p0 + 32, gc0:gc1], 0)
                g_tiles = []
                for b in range(NBANK):
                    nseg = int(nch[g, b])
                    if nseg == 0:
                        g_tiles.append(None)
                        continue
                    g_t = gpool.tile([P, nseg, P], mybir.dt.float8e3,
                                     tag=f"g{b}")
                    rows = min(BANK, cfg.NV - b * BANK)
                    off = int(seg_base[g, b]) // 16
                    nc.sync.dma_start(out=idx_t[0:32, off:off + nseg * 8],
                                      in_=t_idx[:, off:off + nseg * 8])
                    _dma_gather_raw(
                        nc.gpsimd, g_t[:, :, :],
                        t_embs8[b * BANK: b * BANK + rows, 0:P],
                        idx_t[:, off:off + nseg * 8],
                        num_idxs=nseg * P, elem_size=P, elem_step=256,
                    )
                    g_tiles.append(g_t)
                if not gx_loaded:
                    # deferred so the first gathers go out first
                    nc.sync.dma_start(out=gx_t, in_=t_gx)
                    gx_loaded = True

                for u0 in range(0, nb, UNIT):
                    uw = min(UNIT, nb - u0)
                    psum_o = psb.tile([P, UNIT * P], mybir.dt.float32,
                                      space="PSUM")
                    for s in range(u0, u0 + uw):
                        cols = sched[g][s]
                        psum_u = psu.tile([P, P], mybir.dt.float32,
                                          space="PSUM")
                        nlast = len(cols) - 1
                        for j, (b, ch, cid) in enumerate(cols):
                            s_ap = get_s(cid)
                            nc.tensor.matmul(
                                out=psum_u[:, :],
                                lhsT=g_tiles[b][:, ch, :],
                                rhs=s_ap,
                                start=(j == 0), stop=(j == nlast),
                            )
                        u16 = xfer.tile([P, P], mybir.dt.float16, tag="u")
                        nc.scalar.activation(
                            out=u16, in_=psum_u,
                            func=mybir.ActivationFunctionType.Copy)
                        q = s - u0
                        nc.tensor.matmul(
                            out=psum_o[:, q * P:(q + 1) * P], lhsT=u16,
                            rhs=w_t, start=True, stop=True,
                            skip_group_check=True)
                    flush_pending()
                    state["pending"] = (psum_o, (st_g + u0) * P, uw * P, g)
            flush_pending()
            if state["wcol"] < npad:
                nc.sync.dma_start(out=t_out[:, state["wcol"]:],
                                  in_=out_t[:, state["wcol"]:])
    nc.compile()
    return nc


def _unshard(cfg, meta, res):
    TPC, NSB = cfg.TPC, cfg.NSB
    outs = []
    for c in range(cfg.NCORES):
        o = res.results[c]["out"]                       # [P, NSB*P] fp16
        o = np.asarray(o, np.float32).reshape(P, NSB, P)
        o = o.transpose(1, 0, 2).reshape(NSB * P, P)[:TPC]
        outs.append(o)
    virt = np.concatenate(outs, axis=0)
    return virt[meta["perm"]].astype(np.float32)


def kernel(X, embs, W, edge_index, edge_weight):
    cfg = _REAL
    meta, in_maps = _host_prep(cfg, X, embs, W, edge_index, edge_weight)
    nc = _build_program(cfg, meta)
    res = run_bass_kernel_spmd(nc, in_maps, list(range(cfg.NCORES)))
    return _unshard(cfg, meta, res)


# revision 64
# speedup vs baseline: 3.1866x; 1.0025x over previous
"""GCNConv-style message passing kernel for Trainium2, 8 NeuronCores.

Reference semantics:
    deg  = 1 + segment_sum(edge_weight, col)           # self-loop included
    dinv = deg ** -0.5
    h    = embs @ W
    out[t] = (sum_e norm_e * h[src_e] + dinv[t]^2 * h[t]) * X[t],
             norm_e = dinv[src_e] * ew_e * dinv[t]

Device formulation (matmul commutes past the segment sum):
    embs8 = e3m4(SCALE * dinv[:, None] * embs)          (host, fp8 e3m4)
    u[t]  = sum_{e: col=t} ew_e * embs8[src_e]          (self loop folded in
                                                         as an extra edge)
    out[t] = (fp16(u[t]) @ W16) * fp16(dinv[t] * X[t] / SCALE)

Sharding: targets split across 8 cores (12500 each). Edges (incl. self
edges, whose sources point into a per-core virtual row region so the
layout is core-uniform) are grouped by (dest-block-group of SBG x 128
targets, source bank of 32768 rows) into slot segments; within a segment,
per-(dest-block, bank) slot spans are sized max-over-cores so the chunk
schedule is shared SPMD. Source rows are fetched with a raw 128-byte fp8
dma_gather (256B DRAM stride). Selection matrices S[e, t_loc] for BATCH
chunks at a time are built by ONE DVE tensor_tensor is_equal (2x fp16
mode) into a column-major [128, 128, BATCH] tile; chunk j's S is the
stride-BATCH slice [:, :, j], consumed by PE matmuls accumulating PSUM
u^T[cin, t_loc]. Chunks straddling dest-block boundaries emit one masked
S column per covered block. ACT copies PSUM->SBUF as fp16, PE applies W
into 4-block-wide PSUM tiles, DVE multiplies by the gating (deferred one
unit to avoid stalls) and the result is written back once as fp16.
"""

import numpy as np
import ml_dtypes

import concourse.bacc as bacc
import concourse.tile as tile
from concourse import mybir
from concourse.bass import exact_div
from concourse.bass_utils import run_bass_kernel_spmd

P = 128
BATCH = 16            # S columns built per DVE instruction
UNIT = 4              # dest blocks per output psum tile
E3M4 = ml_dtypes.float8_e3m4
SCALE = 4.0
E3M4_MAX = 15.5


class _Cfg:
    def __init__(self, n, n_cores, sb_group):
        self.N = n
        self.NCORES = n_cores
        self.TPC = n // n_cores              # targets per core
        assert self.TPC * n_cores == n
        self.NSB = -(-self.TPC // P)         # dest blocks of 128 per core
        self.BANK = 32768                    # gather bank rows (int16 idx)
        self.NV = n + self.TPC               # rows incl. per-core self region
        self.NBANK = -(-self.NV // self.BANK)
        # taper the last groups so the end-of-pipeline drain is short
        tail = [4, 3, 2, 1]
        sizes = []
        rem = self.NSB - sum(tail)
        if rem < sb_group:
            tail, rem = [], self.NSB
        while rem >= sb_group:
            sizes.append(sb_group)
            rem -= sb_group
        if rem:
            sizes.append(rem)
        sizes += tail
        assert sum(sizes) == self.NSB
        self.GROUPS = []                     # (start_block, nblocks)
        s0 = 0
        for sz in sizes:
            self.GROUPS.append((s0, sz))
            s0 += sz
        self.NG = len(self.GROUPS)
        self.SBGMAX = max(sz for _, sz in self.GROUPS)
        # block -> (group, local index)
        self.G_OF_SB = np.zeros(self.NSB, np.int64)
        self.SBL_OF_SB = np.zeros(self.NSB, np.int64)
        for g, (st, sz) in enumerate(self.GROUPS):
            self.G_OF_SB[st:st + sz] = g
            self.SBL_OF_SB[st:st + sz] = np.arange(sz)


_REAL = _Cfg(n=100000, n_cores=8, sb_group=8)


def _dma_gather_raw(gp, out_ap, in_ap, idxs_ap, num_idxs, elem_size,
                    elem_step, single_packet=False):
    """bass dma_gather clone (DRAM src, non-transpose) without the
    elem_size%256B restriction; elem_step sets the 256B-unit DRAM stride."""
    assert idxs_ap.dtype == mybir.dt.int16
    assert in_ap.dtype == out_ap.dtype
    assert in_ap.ap[-1][1] == elem_size
    assert out_ap.ap[-1][1] == elem_size
    assert out_ap.ap[0][1] * out_ap.ap[1][1] == ((num_idxs + 127) // 128) * 128
    assert in_ap.ap[0][0] == elem_step
    stride_bytes_256 = exact_div(elem_step * mybir.dt.size(in_ap.dtype), 256)
    assert 0 < stride_bytes_256 < 256
    _in_ap = gp.lower_ap_dma(in_ap, for_custom_bir_dma=True)
    _idxs_ap = gp.lower_ap(idxs_ap)
    _out_ap = gp.lower_ap(out_ap)
    return gp.add_instruction(
        mybir.InstDMAGatherAnt(
            name=gp.bass.get_next_instruction_name(),
            ins=[*_in_ap, _idxs_ap, gp.lower_val_access(gp.to_reg(num_idxs))],
            outs=[_out_ap],
            transpose=False,
            num_idxs=num_idxs,
            elem_size=elem_size,
            stride_bytes_256=stride_bytes_256,
            gen_mode=0,
            single_packet=single_packet,
            queue_num=0,
            sbuf_tokens_per_rank=0,
            sbuf_free_dim_per_rank=0,
            sbuf_free_dim_pad_per_rank=0,
            sbuf_byte_offset=0,
        )
    )


def _bcast_mid(ap, rep):
    """[P, k] AP -> [P, rep, k] with a stride-0 middle dim."""
    cls = type(ap)
    new = [list(ap.ap[0]), [0, rep], list(ap.ap[1])]
    return cls(ap.tensor, ap.offset, new)


def _as3d(ap, mid, last):
    """[P, mid*last] contiguous AP -> [P, mid, last] view."""
    cls = type(ap)
    assert list(ap.ap[1]) == [1, mid * last]
    new = [list(ap.ap[0]), [last, mid], [1, last]]
    return cls(ap.tensor, ap.offset, new)


def _balance_perm(cfg, src, col):
    """Greedy target -> virtual-id assignment equalizing per-(block
    position, source bank) in-degree across cores, which shrinks the
    max-over-cores slot padding. perm[t] = virtual id."""
    N, TPC, NSB, BANK, NBANK = cfg.N, cfg.TPC, cfg.NSB, cfg.BANK, cfg.NBANK
    NC = cfg.NCORES
    dkey = col * NBANK + src // BANK
    d = np.bincount(dkey, minlength=N * NBANK).reshape(N, NBANK)
    tot = d.sum(1)
    order = np.argsort(-tot, kind="stable")
    perm = np.empty(N, np.int64)
    pos = 0
    for k in range(NSB):
        blkN = min(P, TPC - k * P)
        cand = order[pos: pos + blkN * NC]
        pos += blkN * NC
        cnt = np.zeros(NC, np.int64)
        R = np.zeros((NC, NBANK), np.int64)
        for t in cand:
            # cost: increase of sum_b max_c R[c,b] (the actual padding),
            # tie-broken toward the least-loaded core
            curmax = R.max(axis=0)
            inc = np.maximum(R + d[t] - curmax, 0).sum(axis=1)
            inc = inc.astype(np.float64) + 1e-3 * R.sum(axis=1)
            inc[cnt >= blkN] = 1e18
            c = int(np.argmin(inc))
            perm[t] = c * TPC + k * P + cnt[c]
            R[c] += d[t]
            cnt[c] += 1
    return perm


def _host_prep(cfg, X, embs, W, edge_index, edge_weight):
    """Build fp8 embs table, slot layout, chunk schedule, per-core arrays."""
    N, TPC, NSB, BANK, NBANK = cfg.N, cfg.TPC, cfg.NSB, cfg.BANK, cfg.NBANK
    NC, SBG, NG = cfg.NCORES, cfg.SBGMAX, cfg.NG

    src = np.asarray(edge_index[0], dtype=np.int64)
    col = np.asarray(edge_index[1], dtype=np.int64)
    ew = np.asarray(edge_weight, dtype=np.float32)
    ew_ones = bool(np.all(ew == 1.0))

    perm = _balance_perm(cfg, src, col)
    inv = np.empty(N, np.int64)
    inv[perm] = np.arange(N)

    # self loops as ordinary edges; their source points into a per-core
    # virtual row region [N, N+TPC) so the (block, bank) slot layout is
    # identical across cores. Targets are remapped through perm.
    loop = np.arange(N, dtype=np.int64)
    src_a = np.concatenate([src, N + (perm[loop] % TPC)])
    col_a = np.concatenate([perm[col], perm[loop]])
    ew_a = np.concatenate([ew, np.ones(N, np.float32)])

    deg = 1.0 + np.bincount(col, weights=ew.astype(np.float64), minlength=N)
    dinv = (1.0 / np.sqrt(deg)).astype(np.float32)

    embs_s = dinv[:, None] * np.asarray(embs, np.float32)
    embs8 = np.zeros((cfg.NV, 256), E3M4)
    embs8[:N, :P] = np.clip(embs_s * SCALE, -E3M4_MAX, E3M4_MAX).astype(E3M4)
    gx = (dinv[:, None] * np.asarray(X, np.float32)) * (1.0 / SCALE)

    # bucket keys
    core = col_a // TPC
    sb = (col_a % TPC) // P                  # 0..NSB-1
    grp = cfg.G_OF_SB[sb]
    sbl = cfg.SBL_OF_SB[sb]                  # block local to group
    bank = src_a // BANK

    key = ((core * NG + grp) * NBANK + bank) * SBG + sbl
    nkey = NC * NG * NBANK * SBG
    counts = np.bincount(key, minlength=nkey).reshape(NC, NG, NBANK, SBG)
    M = counts.max(axis=0)                   # shared span sizes [NG,NBANK,SBG]

    # shared slot layout: segments (g,b) in order, blocks in order inside,
    # each segment padded to a 128 multiple.
    blk_off = np.zeros((NG, NBANK, SBG), np.int64)   # block span start
    seg_base = np.zeros((NG, NBANK), np.int64)       # segment slot base
    nch = np.zeros((NG, NBANK), np.int64)            # chunks per segment
    seg_len = np.zeros((NG, NBANK), np.int64)        # real slots (16-aligned)
    pos = 0
    for g in range(NG):
        nb = cfg.GROUPS[g][1]
        for b in range(NBANK):
            seg_base[g, b] = pos
            off = 0
            for s in range(nb):
                blk_off[g, b, s] = off
                off += M[g, b, s]
            nch[g, b] = -(-off // P)
            seg_len[g, b] = -(-off // 16) * 16
            pos += nch[g, b] * P
    slots_tot = pos

    # chunk/block overlap -> tloc columns, in device processing order
    colid = {}
    sched = []                               # [g][s_local] -> [(b, ch, col)]
    ncols = 0
    for g in range(NG):
        nb = cfg.GROUPS[g][1]
        gsched = []
        for s in range(nb):
            lst = []
            for b in range(NBANK):
                lo = blk_off[g, b, s]
                hi = lo + M[g, b, s]
                if hi == lo:
                    continue
                c0, c1 = lo // P, (hi - 1) // P
                for c in range(c0, c1 + 1):
                    colid[(g, b, c, s)] = ncols
                    lst.append((b, int(c), ncols))
                    ncols += 1
            assert lst, f"block {g},{s} has no slots"
            gsched.append(lst)
        sched.append(gsched)
    ncols_pad = -(-ncols // BATCH) * BATCH

    # per-edge destination slots
    cnt_flat = counts.reshape(-1)
    order = np.argsort(key, kind="stable")
    starts = np.zeros_like(cnt_flat)
    np.cumsum(cnt_flat[:-1], out=starts[1:])
    k_sorted = key[order]
    rank = np.arange(len(order)) - starts[k_sorted]
    g_s, b_s = grp[order], bank[order]
    sbl_s, core_s = sbl[order], core[order]
    dest = (seg_base[g_s, b_s] + blk_off[g_s, b_s, sbl_s] + rank)
    chunk_s = (dest - seg_base[g_s, b_s]) // P    # segment-local chunk
    lane_s = dest % P
    tl_s = (col_a[order] % TPC % P).astype(np.float32)
    src_l = (src_a[order] - b_s * BANK).astype(np.int16)
    ew_s = ew_a[order]

    # column index per edge (vectorized via dict -> array)
    mchunk = slots_tot // P + 1
    ckey = ((g_s * NBANK + b_s) * mchunk + chunk_s) * SBG + sbl_s
    uk, uinv = np.unique(ckey, return_inverse=True)
    uk_col = np.empty(len(uk), np.int64)
    for i, k in enumerate(uk):
        sblk = k % SBG
        k //= SBG
        ch = k % mchunk
        k //= mchunk
        b = k % NBANK
        g = k // NBANK
        uk_col[i] = colid[(g, b, ch, sblk)]
    col_e = uk_col[uinv]

    IDX = np.zeros((NC, slots_tot), np.int16)
    TLOC = np.full((NC, P, ncols_pad), -100.0, np.float16)
    IDX[core_s, dest] = src_l
    TLOC[core_s, lane_s, col_e] = tl_s
    EWC = None
    if not ew_ones:
        EWC = np.ones((NC, P, ncols_pad), np.float16)
        EWC[core_s, lane_s, col_e] = ew_s

    # pack gather indices wrap-16; the Q7 gather ucode only reads idx
    # partitions 0-31 (cores 0-1 generate all descriptors), so replicate to
    # 32 partitions only; the device zero-fills partitions 32-127.
    idx_packed = IDX.reshape(NC, slots_tot // 16, 16).transpose(0, 2, 1)
    idx_all = np.tile(idx_packed, (1, 2, 1)).astype(np.int16)

    # partition-major gx layout in virtual row order
    npad = NSB * P
    gx_v = gx[inv]
    gx_pm = np.zeros((NC, P, npad), np.float16)
    for c in range(NC):
        gxc = gx_v[c * TPC:(c + 1) * TPC]
        gxc = np.concatenate(
            [gxc, np.zeros((npad - TPC, P), np.float32)], axis=0)
        gx_pm[c] = gxc.reshape(NSB, P, P).transpose(1, 0, 2).reshape(
            P, npad).astype(np.float16)

    # iotacm[p, t*BATCH + j] = t  (column-major S layout constant)
    iotacm = np.repeat(np.arange(P, dtype=np.float16), BATCH)[None, :]
    iotacm = np.tile(iotacm, (P, 1))
    w16 = np.asarray(W, np.float16)

    meta = dict(sched=sched, nch=nch, seg_base=seg_base, slots_tot=slots_tot,
                ncols=ncols, ncols_pad=ncols_pad, ew_ones=ew_ones, perm=perm)
    in_maps = []
    for c in range(NC):
        e8 = embs8.copy()
        e8[N:N + TPC] = embs8[inv[c * TPC:(c + 1) * TPC]]
        m = dict(
            embs8=e8,
            w16=w16,
            gx=np.ascontiguousarray(gx_pm[c]),
            idxall=np.ascontiguousarray(idx_all[c]),
            tlocall=np.ascontiguousarray(TLOC[c].reshape(P, ncols_pad)),
            iotacm=iotacm,
        )
        if EWC is not None:
            m["ewall"] = np.ascontiguousarray(EWC[c].reshape(P, ncols_pad))
        in_maps.append(m)
    return meta, in_maps


def _build_program(cfg, meta):
    N, TPC, NSB, BANK, NBANK = cfg.N, cfg.TPC, cfg.NSB, cfg.BANK, cfg.NBANK
    NG = cfg.NG
    sched, nch, seg_base = meta["sched"], meta["nch"], meta["seg_base"]
    slots_tot, ncols_pad, ew_ones = (meta["slots_tot"], meta["ncols_pad"],
                                     meta["ew_ones"])
    npad = NSB * P

    nc = bacc.Bacc("TRN2", target_bir_lowering=False, debug=False,
                   num_devices=cfg.NCORES)
    t_embs8 = nc.dram_tensor("embs8", [cfg.NV, 256], mybir.dt.float8e3,
                             kind="ExternalInput").ap()
    t_w = nc.dram_tensor("w16", [P, P], mybir.dt.float16,
                         kind="ExternalInput").ap()
    t_gx = nc.dram_tensor("gx", [P, npad], mybir.dt.float16,
                          kind="ExternalInput").ap()
    t_idx = nc.dram_tensor("idxall", [32, slots_tot // 16], mybir.dt.int16,
                           kind="ExternalInput").ap()
    t_tloc = nc.dram_tensor("tlocall", [P, ncols_pad], mybir.dt.float16,
                            kind="ExternalInput").ap()
    t_iotacm = nc.dram_tensor("iotacm", [P, P * BATCH], mybir.dt.float16,
                              kind="ExternalInput").ap()
    t_ew = None
    if not ew_ones:
        t_ew = nc.dram_tensor("ewall", [P, ncols_pad], mybir.dt.float16,
                              kind="ExternalInput").ap()
    t_out = nc.dram_tensor("out", [P, npad], mybir.dt.float16,
                           kind="ExternalOutput").ap()

    with tile.TileContext(nc) as tc:
        with tc.tile_pool(name="const", bufs=1) as cpool, \
             tc.tile_pool(name="gpool", bufs=3) as gpool, \
             tc.tile_pool(name="spool", bufs=6) as spool, \
             tc.tile_pool(name="xfer", bufs=4) as xfer, \
             tc.tile_pool(name="psu", bufs=4, space="PSUM") as psu, \
             tc.tile_pool(name="psb", bufs=3, space="PSUM") as psb:

            tloc_t = cpool.tile([P, ncols_pad], mybir.dt.float16)
            nc.sync.dma_start(out=tloc_t, in_=t_tloc)
            iotacm_t = cpool.tile([P, P * BATCH], mybir.dt.float16)
            nc.sync.dma_start(out=iotacm_t, in_=t_iotacm)
            w_t = cpool.tile([P, P], mybir.dt.float16)
            nc.sync.dma_start(out=w_t, in_=t_w)
            idx_t = cpool.tile([P, slots_tot // 16], mybir.dt.int16)
            ew_t = None
            if t_ew is not None:
                ew_t = cpool.tile([P, ncols_pad], mybir.dt.float16)
                nc.sync.dma_start(out=ew_t, in_=t_ew)
            gx_t = cpool.tile([P, npad], mybir.dt.float16)
            out_t = cpool.tile([P, npad], mybir.dt.float16)
            gx_loaded = False

            state = dict(batch=-1, s_cm=None, pending=None, wrote=0, wcol=0)

            def get_s(cid):
                bi = cid // BATCH
                if bi != state["batch"]:
                    s_cm = spool.tile([P, P, BATCH], mybir.dt.float16,
                                      tag="s")
                    nc.vector.tensor_tensor(
                        out=s_cm[:, :, :],
                        in0=_as3d(iotacm_t[:, :], P, BATCH),
                        in1=_bcast_mid(tloc_t[:, bi * BATCH:(bi + 1) * BATCH],
                                       P),
                        op=mybir.AluOpType.is_equal,
                    )
                    if ew_t is not None:
                        s2 = spool.tile([P, P, BATCH], mybir.dt.float16,
                                        tag="s2")
                        nc.vector.tensor_tensor(
                            out=s2[:, :, :], in0=s_cm[:, :, :],
                            in1=_bcast_mid(
                                ew_t[:, bi * BATCH:(bi + 1) * BATCH], P),
                            op=mybir.AluOpType.mult,
                        )
                        s_cm = s2
                    state["batch"] = bi
                    state["s_cm"] = s_cm
                return state["s_cm"][:, :, cid % BATCH]

            def flush_pending():
                if state["pending"] is None:
                    return
                po, sg, w, gtag = state["pending"]
                nc.vector.tensor_tensor(
                    out=out_t[:, sg:sg + w], in0=po[:, :w],
                    in1=gx_t[:, sg:sg + w], op=mybir.AluOpType.mult)
                state["pending"] = None
                if state["wrote"] < gtag:        # groups < gtag are complete
                    st_h, nb_h = cfg.GROUPS[gtag - 1]
                    c1 = (st_h + nb_h) * P
                    nc.sync.dma_start(out=t_out[:, state["wcol"]:c1],
                                      in_=out_t[:, state["wcol"]:c1])
                    state["wcol"] = c1
                    state["wrote"] = gtag

            for g in range(NG):
                st_g, nb = cfg.GROUPS[g]
                # zero partitions 32-127 of this group's idx column span (the
                # Q7 gather ucode reads only partitions 0-31; the rest just
                # needs defined, in-range values). Done on the idle ACT
                # engine: Copy with scale=0 of any resident fp16 tile.
                gc0 = int(seg_base[g, 0]) // 16
                gc1 = int(seg_base[g, NBANK - 1] + nch[g, NBANK - 1] * P) // 16
                if gc1 > gc0:
                    # zero on the idle ACT engine (Copy with scale=0); ops
                    # with base partition > 0 are limited to 32 partitions
                    assert gc1 - gc0 <= P * BATCH
                    for p0 in (32, 64, 96):
                        nc.scalar.activation(
                            out=idx_t[p0:p0 + 32, gc0:gc1].bitcast(
                                mybir.dt.float16),
                            in_=iotacm_t[p0:p0 + 32, 0:gc1 - gc0],
                            func=mybir.ActivationFunctionType.Copy,
                            scale=0.0)
                g_tiles = []
                for b in range(NBANK):
                    nseg = int(nch[g, b])
                    if nseg == 0:
                        g_tiles.append(None)
                        continue
                    g_t = gpool.tile([P, nseg, P], mybir.dt.float8e3,
                                     tag=f"g{b}")
                    rows = min(BANK, cfg.NV - b * BANK)
                    off = int(seg_base[g, b]) // 16
                    nc.sync.dma_start(out=idx_t[0:32, off:off + nseg * 8],
                                      in_=t_idx[:, off:off + nseg * 8])
                    _dma_gather_raw(
                        nc.gpsimd, g_t[:, :, :],
                        t_embs8[b * BANK: b * BANK + rows, 0:P],
                        idx_t[:, off:off + nseg * 8],
                        num_idxs=nseg * P, elem_size=P, elem_step=256,
                    )
                    g_tiles.append(g_t)
                if not gx_loaded:
                    # deferred so the first gathers go out first
                    nc.sync.dma_start(out=gx_t, in_=t_gx)
                    gx_loaded = True

                for u0 in range(0, nb, UNIT):
                    uw = min(UNIT, nb - u0)
                    psum_o = psb.tile([P, UNIT * P], mybir.dt.float32,
                                      space="PSUM")
                    for s in range(u0, u0 + uw):
                        cols = sched[g][s]
                        psum_u = psu.tile([P, P], mybir.dt.float32,
                                          space="PSUM")
                        nlast = len(cols) - 1
                        for j, (b, ch, cid) in enumerate(cols):
                            s_ap = get_s(cid)
                            nc.tensor.matmul(
                                out=psum_u[:, :],
                                lhsT=g_tiles[b][:, ch, :],
                                rhs=s_ap,
                                start=(j == 0), stop=(j == nlast),
                            )
                        u16 = xfer.tile([P, P], mybir.dt.float16, tag="u")
                        nc.scalar.activation(
                            out=u16, in_=psum_u,
                            func=mybir.ActivationFunctionType.Copy)
                        q = s - u0
                        nc.tensor.matmul(
                            out=psum_o[:, q * P:(q + 1) * P], lhsT=u16,
                            rhs=w_t, start=True, stop=True,
                            skip_group_check=True)
                    flush_pending()
                    state["pending"] = (psum_o, (st_g + u0) * P, uw * P, g)
            flush_pending()
            if state["wcol"] < npad:
                nc.sync.dma_start(out=t_out[:, state["wcol"]:],
                                  in_=out_t[:, state["wcol"]:])
    nc.compile()
    return nc


def _unshard(cfg, meta, res):
    TPC, NSB = cfg.TPC, cfg.NSB
    outs = []
    for c in range(cfg.NCORES):
        o = res.results[c]["out"]                       # [P, NSB*P] fp16
        o = np.asarray(o, np.float32).reshape(P, NSB, P)
        o = o.transpose(1, 0, 2).reshape(NSB * P, P)[:TPC]
        outs.append(o)
    virt = np.concatenate(outs, axis=0)
    return virt[meta["perm"]].astype(np.float32)


def kernel(X, embs, W, edge_index, edge_weight):
    cfg = _REAL
    meta, in_maps = _host_prep(cfg, X, embs, W, edge_index, edge_weight)
    nc = _build_program(cfg, meta)
    res = run_bass_kernel_spmd(nc, in_maps, list(range(cfg.NCORES)))
    return _unshard(cfg, meta, res)


# revision 66
# speedup vs baseline: 3.2465x; 1.0188x over previous
"""GCNConv-style message passing kernel for Trainium2, 8 NeuronCores.

Reference semantics:
    deg  = 1 + segment_sum(edge_weight, col)           # self-loop included
    dinv = deg ** -0.5
    h    = embs @ W
    out[t] = (sum_e norm_e * h[src_e] + dinv[t]^2 * h[t]) * X[t],
             norm_e = dinv[src_e] * ew_e * dinv[t]

Device formulation (matmul commutes past the segment sum):
    embs8 = e3m4(SCALE * dinv[:, None] * embs)          (host, fp8 e3m4)
    u[t]  = sum_{e: col=t} ew_e * embs8[src_e]          (self loop folded in
                                                         as an extra edge)
    out[t] = (fp16(u[t]) @ W16) * fp16(dinv[t] * X[t] / SCALE)

Sharding: targets split across 8 cores (12500 each), permuted by a greedy
balancer so per-(block, source-bank) in-degree is even across cores (this
minimizes the max-over-cores slot padding of the shared SPMD schedule).
Edges (incl. self edges, whose sources point into a per-core virtual row
region so the layout is core-uniform) are grouped by (dest-block group,
source bank of 32768 rows) into slot segments; per-(dest-block, bank)
slot spans are sized max-over-cores. Source rows are fetched with a raw
128-byte fp8 dma_gather (256B DRAM stride; half the cost of a >=256B
descriptor). Gather indices are wrap-16 packed and replicated to only 32
partitions (the Q7 ucode reads idx partitions 0-31; partitions 32-127
are zero-filled by ACT scale=0 copies in 32-partition pieces). Selection
matrices S[e, t_loc] for BATCH chunks at a time are built by ONE DVE
tensor_tensor is_equal (2x fp16 mode) into a column-major
[128, 128, BATCH] tile; chunk j's S is the stride-BATCH slice [:, :, j],
consumed by PE matmuls accumulating PSUM u^T[cin, t_loc]. Chunks
straddling dest-block boundaries emit one masked S column per covered
block. ACT copies PSUM->SBUF as fp16, PE applies W into UNIT-block-wide
PSUM tiles, DVE multiplies by the gating (deferred one unit to avoid
stalls) and group results are written back as fp16. Group sizes taper at
the end to shorten the pipeline drain.
"""

import numpy as np
import ml_dtypes

import concourse.bacc as bacc
import concourse.tile as tile
from concourse import mybir
from concourse.bass import exact_div
from concourse.bass_utils import run_bass_kernel_spmd

P = 128
BATCH = 16            # S columns built per DVE instruction
UNIT = 8              # dest blocks per output psum tile
E3M4 = ml_dtypes.float8_e3m4
SCALE = 4.0
E3M4_MAX = 15.5


class _Cfg:
    def __init__(self, n, n_cores, sb_group):
        self.N = n
        self.NCORES = n_cores
        self.TPC = n // n_cores              # targets per core
        assert self.TPC * n_cores == n
        self.NSB = -(-self.TPC // P)         # dest blocks of 128 per core
        self.BANK = 32768                    # gather bank rows (int16 idx)
        self.NV = n + self.TPC               # rows incl. per-core self region
        self.NBANK = -(-self.NV // self.BANK)
        # taper the last groups so the end-of-pipeline drain is short
        tail = [4, 3, 2, 1]
        sizes = []
        rem = self.NSB - sum(tail)
        if rem < sb_group:
            tail, rem = [], self.NSB
        while rem >= sb_group:
            sizes.append(sb_group)
            rem -= sb_group
        if rem:
            sizes.append(rem)
        sizes += tail
        assert sum(sizes) == self.NSB
        self.GROUPS = []                     # (start_block, nblocks)
        s0 = 0
        for sz in sizes:
            self.GROUPS.append((s0, sz))
            s0 += sz
        self.NG = len(self.GROUPS)
        self.SBGMAX = max(sz for _, sz in self.GROUPS)
        # block -> (group, local index)
        self.G_OF_SB = np.zeros(self.NSB, np.int64)
        self.SBL_OF_SB = np.zeros(self.NSB, np.int64)
        for g, (st, sz) in enumerate(self.GROUPS):
            self.G_OF_SB[st:st + sz] = g
            self.SBL_OF_SB[st:st + sz] = np.arange(sz)


_REAL = _Cfg(n=100000, n_cores=8, sb_group=8)


def _dma_gather_raw(gp, out_ap, in_ap, idxs_ap, num_idxs, elem_size,
                    elem_step, single_packet=False):
    """bass dma_gather clone (DRAM src, non-transpose) without the
    elem_size%256B restriction; elem_step sets the 256B-unit DRAM stride."""
    assert idxs_ap.dtype == mybir.dt.int16
    assert in_ap.dtype == out_ap.dtype
    assert in_ap.ap[-1][1] == elem_size
    assert out_ap.ap[-1][1] == elem_size
    assert out_ap.ap[0][1] * out_ap.ap[1][1] == ((num_idxs + 127) // 128) * 128
    assert in_ap.ap[0][0] == elem_step
    stride_bytes_256 = exact_div(elem_step * mybir.dt.size(in_ap.dtype), 256)
    assert 0 < stride_bytes_256 < 256
    _in_ap = gp.lower_ap_dma(in_ap, for_custom_bir_dma=True)
    _idxs_ap = gp.lower_ap(idxs_ap)
    _out_ap = gp.lower_ap(out_ap)
    return gp.add_instruction(
        mybir.InstDMAGatherAnt(
            name=gp.bass.get_next_instruction_name(),
            ins=[*_in_ap, _idxs_ap, gp.lower_val_access(gp.to_reg(num_idxs))],
            outs=[_out_ap],
            transpose=False,
            num_idxs=num_idxs,
            elem_size=elem_size,
            stride_bytes_256=stride_bytes_256,
            gen_mode=0,
            single_packet=single_packet,
            queue_num=0,
            sbuf_tokens_per_rank=0,
            sbuf_free_dim_per_rank=0,
            sbuf_free_dim_pad_per_rank=0,
            sbuf_byte_offset=0,
        )
    )


def _bcast_mid(ap, rep):
    """[P, k] AP -> [P, rep, k] with a stride-0 middle dim."""
    cls = type(ap)
    new = [list(ap.ap[0]), [0, rep], list(ap.ap[1])]
    return cls(ap.tensor, ap.offset, new)


def _as3d(ap, mid, last):
    """[P, mid*last] contiguous AP -> [P, mid, last] view."""
    cls = type(ap)
    assert list(ap.ap[1]) == [1, mid * last]
    new = [list(ap.ap[0]), [last, mid], [1, last]]
    return cls(ap.tensor, ap.offset, new)


def _balance_perm(cfg, src, col):
    """Greedy target -> virtual-id assignment equalizing per-(block
    position, source bank) in-degree across cores, which shrinks the
    max-over-cores slot padding. perm[t] = virtual id."""
    N, TPC, NSB, BANK, NBANK = cfg.N, cfg.TPC, cfg.NSB, cfg.BANK, cfg.NBANK
    NC = cfg.NCORES
    dkey = col * NBANK + src // BANK
    d = np.bincount(dkey, minlength=N * NBANK).reshape(N, NBANK)
    tot = d.sum(1)
    order = np.argsort(-tot, kind="stable")
    perm = np.empty(N, np.int64)
    pos = 0
    for k in range(NSB):
        blkN = min(P, TPC - k * P)
        cand = order[pos: pos + blkN * NC]
        pos += blkN * NC
        cnt = np.zeros(NC, np.int64)
        R = np.zeros((NC, NBANK), np.int64)
        for t in cand:
            # cost: increase of sum_b max_c R[c,b] (the actual padding),
            # tie-broken toward the least-loaded core
            curmax = R.max(axis=0)
            inc = np.maximum(R + d[t] - curmax, 0).sum(axis=1)
            inc = inc.astype(np.float64) + 1e-3 * R.sum(axis=1)
            inc[cnt >= blkN] = 1e18
            c = int(np.argmin(inc))
            perm[t] = c * TPC + k * P + cnt[c]
            R[c] += d[t]
            cnt[c] += 1
    return perm


def _host_prep(cfg, X, embs, W, edge_index, edge_weight):
    """Build fp8 embs table, slot layout, chunk schedule, per-core arrays."""
    N, TPC, NSB, BANK, NBANK = cfg.N, cfg.TPC, cfg.NSB, cfg.BANK, cfg.NBANK
    NC, SBG, NG = cfg.NCORES, cfg.SBGMAX, cfg.NG

    src = np.asarray(edge_index[0], dtype=np.int64)
    col = np.asarray(edge_index[1], dtype=np.int64)
    ew = np.asarray(edge_weight, dtype=np.float32)
    ew_ones = bool(np.all(ew == 1.0))

    perm = _balance_perm(cfg, src, col)
    inv = np.empty(N, np.int64)
    inv[perm] = np.arange(N)

    # self loops as ordinary edges; their source points into a per-core
    # virtual row region [N, N+TPC) so the (block, bank) slot layout is
    # identical across cores. Targets are remapped through perm.
    loop = np.arange(N, dtype=np.int64)
    src_a = np.concatenate([src, N + (perm[loop] % TPC)])
    col_a = np.concatenate([perm[col], perm[loop]])
    ew_a = np.concatenate([ew, np.ones(N, np.float32)])

    deg = 1.0 + np.bincount(col, weights=ew.astype(np.float64), minlength=N)
    dinv = (1.0 / np.sqrt(deg)).astype(np.float32)

    embs_s = dinv[:, None] * np.asarray(embs, np.float32)
    embs8 = np.zeros((cfg.NV, 256), E3M4)
    embs8[:N, :P] = np.clip(embs_s * SCALE, -E3M4_MAX, E3M4_MAX).astype(E3M4)
    gx = (dinv[:, None] * np.asarray(X, np.float32)) * (1.0 / SCALE)

    # bucket keys
    core = col_a // TPC
    sb = (col_a % TPC) // P                  # 0..NSB-1
    grp = cfg.G_OF_SB[sb]
    sbl = cfg.SBL_OF_SB[sb]                  # block local to group
    bank = src_a // BANK

    key = ((core * NG + grp) * NBANK + bank) * SBG + sbl
    nkey = NC * NG * NBANK * SBG
    counts = np.bincount(key, minlength=nkey).reshape(NC, NG, NBANK, SBG)
    M = counts.max(axis=0)                   # shared span sizes [NG,NBANK,SBG]

    # shared slot layout: segments (g,b) in order, blocks in order inside,
    # each segment padded to a 128 multiple.
    blk_off = np.zeros((NG, NBANK, SBG), np.int64)   # block span start
    seg_base = np.zeros((NG, NBANK), np.int64)       # segment slot base
    nch = np.zeros((NG, NBANK), np.int64)            # chunks per segment
    seg_len = np.zeros((NG, NBANK), np.int64)        # real slots (16-aligned)
    pos = 0
    for g in range(NG):
        nb = cfg.GROUPS[g][1]
        for b in range(NBANK):
            seg_base[g, b] = pos
            off = 0
            for s in range(nb):
                blk_off[g, b, s] = off
                off += M[g, b, s]
            nch[g, b] = -(-off // P)
            seg_len[g, b] = -(-off // 16) * 16
            pos += nch[g, b] * P
    slots_tot = pos

    # chunk/block overlap -> tloc columns, in device processing order
    colid = {}
    sched = []                               # [g][s_local] -> [(b, ch, col)]
    ncols = 0
    for g in range(NG):
        nb = cfg.GROUPS[g][1]
        gsched = []
        for s in range(nb):
            lst = []
            for b in range(NBANK):
                lo = blk_off[g, b, s]
                hi = lo + M[g, b, s]
                if hi == lo:
                    continue
                c0, c1 = lo // P, (hi - 1) // P
                for c in range(c0, c1 + 1):
                    colid[(g, b, c, s)] = ncols
                    lst.append((b, int(c), ncols))
                    ncols += 1
            assert lst, f"block {g},{s} has no slots"
            gsched.append(lst)
        sched.append(gsched)
    ncols_pad = -(-ncols // BATCH) * BATCH

    # per-edge destination slots
    cnt_flat = counts.reshape(-1)
    order = np.argsort(key, kind="stable")
    starts = np.zeros_like(cnt_flat)
    np.cumsum(cnt_flat[:-1], out=starts[1:])
    k_sorted = key[order]
    rank = np.arange(len(order)) - starts[k_sorted]
    g_s, b_s = grp[order], bank[order]
    sbl_s, core_s = sbl[order], core[order]
    dest = (seg_base[g_s, b_s] + blk_off[g_s, b_s, sbl_s] + rank)
    chunk_s = (dest - seg_base[g_s, b_s]) // P    # segment-local chunk
    lane_s = dest % P
    tl_s = (col_a[order] % TPC % P).astype(np.float32)
    src_l = (src_a[order] - b_s * BANK).astype(np.int16)
    ew_s = ew_a[order]

    # column index per edge (vectorized via dict -> array)
    mchunk = slots_tot // P + 1
    ckey = ((g_s * NBANK + b_s) * mchunk + chunk_s) * SBG + sbl_s
    uk, uinv = np.unique(ckey, return_inverse=True)
    uk_col = np.empty(len(uk), np.int64)
    for i, k in enumerate(uk):
        sblk = k % SBG
        k //= SBG
        ch = k % mchunk
        k //= mchunk
        b = k % NBANK
        g = k // NBANK
        uk_col[i] = colid[(g, b, ch, sblk)]
    col_e = uk_col[uinv]

    IDX = np.zeros((NC, slots_tot), np.int16)
    TLOC = np.full((NC, P, ncols_pad), -100.0, np.float16)
    IDX[core_s, dest] = src_l
    TLOC[core_s, lane_s, col_e] = tl_s
    EWC = None
    if not ew_ones:
        EWC = np.ones((NC, P, ncols_pad), np.float16)
        EWC[core_s, lane_s, col_e] = ew_s

    # pack gather indices wrap-16; the Q7 gather ucode only reads idx
    # partitions 0-31 (cores 0-1 generate all descriptors), so replicate to
    # 32 partitions only; the device zero-fills partitions 32-127.
    idx_packed = IDX.reshape(NC, slots_tot // 16, 16).transpose(0, 2, 1)
    idx_all = np.tile(idx_packed, (1, 2, 1)).astype(np.int16)

    # partition-major gx layout in virtual row order
    npad = NSB * P
    gx_v = gx[inv]
    gx_pm = np.zeros((NC, P, npad), np.float16)
    for c in range(NC):
        gxc = gx_v[c * TPC:(c + 1) * TPC]
        gxc = np.concatenate(
            [gxc, np.zeros((npad - TPC, P), np.float32)], axis=0)
        gx_pm[c] = gxc.reshape(NSB, P, P).transpose(1, 0, 2).reshape(
            P, npad).astype(np.float16)

    # iotacm[p, t*BATCH + j] = t  (column-major S layout constant)
    iotacm = np.repeat(np.arange(P, dtype=np.float16), BATCH)[None, :]
    iotacm = np.tile(iotacm, (P, 1))
    w16 = np.asarray(W, np.float16)

    meta = dict(sched=sched, nch=nch, seg_base=seg_base, slots_tot=slots_tot,
                ncols=ncols, ncols_pad=ncols_pad, ew_ones=ew_ones, perm=perm)
    in_maps = []
    for c in range(NC):
        e8 = embs8.copy()
        e8[N:N + TPC] = embs8[inv[c * TPC:(c + 1) * TPC]]
        m = dict(
            embs8=e8,
            w16=w16,
            gx=np.ascontiguousarray(gx_pm[c]),
            idxall=np.ascontiguousarray(idx_all[c]),
            tlocall=np.ascontiguousarray(TLOC[c].reshape(P, ncols_pad)),
            iotacm=iotacm,
        )
        if EWC is not None:
            m["ewall"] = np.ascontiguousarray(EWC[c].reshape(P, ncols_pad))
        in_maps.append(m)
    return meta, in_maps


def _build_program(cfg, meta):
    N, TPC, NSB, BANK, NBANK = cfg.N, cfg.TPC, cfg.NSB, cfg.BANK, cfg.NBANK
    NG = cfg.NG
    sched, nch, seg_base = meta["sched"], meta["nch"], meta["seg_base"]
    slots_tot, ncols_pad, ew_ones = (meta["slots_tot"], meta["ncols_pad"],
                                     meta["ew_ones"])
    npad = NSB * P

    nc = bacc.Bacc("TRN2", target_bir_lowering=False, debug=False,
                   num_devices=cfg.NCORES)
    t_embs8 = nc.dram_tensor("embs8", [cfg.NV, 256], mybir.dt.float8e3,
                             kind="ExternalInput").ap()
    t_w = nc.dram_tensor("w16", [P, P], mybir.dt.float16,
                         kind="ExternalInput").ap()
    t_gx = nc.dram_tensor("gx", [P, npad], mybir.dt.float16,
                          kind="ExternalInput").ap()
    t_idx = nc.dram_tensor("idxall", [32, slots_tot // 16], mybir.dt.int16,
                           kind="ExternalInput").ap()
    t_tloc = nc.dram_tensor("tlocall", [P, ncols_pad], mybir.dt.float16,
                            kind="ExternalInput").ap()
    t_iotacm = nc.dram_tensor("iotacm", [P, P * BATCH], mybir.dt.float16,
                              kind="ExternalInput").ap()
    t_ew = None
    if not ew_ones:
        t_ew = nc.dram_tensor("ewall", [P, ncols_pad], mybir.dt.float16,
                              kind="ExternalInput").ap()
    t_out = nc.dram_tensor("out", [P, npad], mybir.dt.float16,
                           kind="ExternalOutput").ap()

    with tile.TileContext(nc) as tc:
        with tc.tile_pool(name="const", bufs=1) as cpool, \
             tc.tile_pool(name="gpool", bufs=3) as gpool, \
             tc.tile_pool(name="spool", bufs=9) as spool, \
             tc.tile_pool(name="xfer", bufs=4) as xfer, \
             tc.tile_pool(name="psu", bufs=4, space="PSUM") as psu, \
             tc.tile_pool(name="psb", bufs=2, space="PSUM") as psb:

            tloc_t = cpool.tile([P, ncols_pad], mybir.dt.float16)
            nc.sync.dma_start(out=tloc_t, in_=t_tloc)
            iotacm_t = cpool.tile([P, P * BATCH], mybir.dt.float16)
            nc.sync.dma_start(out=iotacm_t, in_=t_iotacm)
            w_t = cpool.tile([P, P], mybir.dt.float16)
            nc.sync.dma_start(out=w_t, in_=t_w)
            idx_t = cpool.tile([P, slots_tot // 16], mybir.dt.int16)
            ew_t = None
            if t_ew is not None:
                ew_t = cpool.tile([P, ncols_pad], mybir.dt.float16)
                nc.sync.dma_start(out=ew_t, in_=t_ew)
            gx_t = cpool.tile([P, npad], mybir.dt.float16)
            out_t = cpool.tile([P, npad], mybir.dt.float16)
            gx_loaded = False

            state = dict(batch=-1, s_cm=None, pending=None, wrote=0, wcol=0)

            def get_s(cid):
                bi = cid // BATCH
                if bi != state["batch"]:
                    s_cm = spool.tile([P, P, BATCH], mybir.dt.float16,
                                      tag="s")
                    nc.vector.tensor_tensor(
                        out=s_cm[:, :, :],
                        in0=_as3d(iotacm_t[:, :], P, BATCH),
                        in1=_bcast_mid(tloc_t[:, bi * BATCH:(bi + 1) * BATCH],
                                       P),
                        op=mybir.AluOpType.is_equal,
                    )
                    if ew_t is not None:
                        s2 = spool.tile([P, P, BATCH], mybir.dt.float16,
                                        tag="s2")
                        nc.vector.tensor_tensor(
                            out=s2[:, :, :], in0=s_cm[:, :, :],
                            in1=_bcast_mid(
                                ew_t[:, bi * BATCH:(bi + 1) * BATCH], P),
                            op=mybir.AluOpType.mult,
                        )
                        s_cm = s2
                    state["batch"] = bi
                    state["s_cm"] = s_cm
                return state["s_cm"][:, :, cid % BATCH]

            def flush_pending():
                if state["pending"] is None:
                    return
                po, sg, w, gtag = state["pending"]
                nc.vector.tensor_tensor(
                    out=out_t[:, sg:sg + w], in0=po[:, :w],
                    in1=gx_t[:, sg:sg + w], op=mybir.AluOpType.mult)
                state["pending"] = None
                if state["wrote"] < gtag:        # groups < gtag are complete
                    st_h, nb_h = cfg.GROUPS[gtag - 1]
                    c1 = (st_h + nb_h) * P
                    nc.sync.dma_start(out=t_out[:, state["wcol"]:c1],
                                      in_=out_t[:, state["wcol"]:c1])
                    state["wcol"] = c1
                    state["wrote"] = gtag

            for g in range(NG):
                st_g, nb = cfg.GROUPS[g]
                # zero partitions 32-127 of this group's idx column span (the
                # Q7 gather ucode reads only partitions 0-31; the rest just
                # needs defined, in-range values). Done on the idle ACT
                # engine: Copy with scale=0 of any resident fp16 tile.
                gc0 = int(seg_base[g, 0]) // 16
                gc1 = int(seg_base[g, NBANK - 1] + nch[g, NBANK - 1] * P) // 16
                if gc1 > gc0:
                    # zero on the idle ACT engine (Copy with scale=0); ops
                    # with base partition > 0 are limited to 32 partitions
                    assert gc1 - gc0 <= P * BATCH
                    for p0 in (32, 64, 96):
                        nc.scalar.activation(
                            out=idx_t[p0:p0 + 32, gc0:gc1].bitcast(
                                mybir.dt.float16),
                            in_=iotacm_t[p0:p0 + 32, 0:gc1 - gc0],
                            func=mybir.ActivationFunctionType.Copy,
                            scale=0.0)
                g_tiles = []
                for b in range(NBANK):
                    nseg = int(nch[g, b])
                    if nseg == 0:
                        g_tiles.append(None)
                        continue
                    g_t = gpool.tile([P, nseg, P], mybir.dt.float8e3,
                                     tag=f"g{b}")
                    rows = min(BANK, cfg.NV - b * BANK)
                    off = int(seg_base[g, b]) // 16
                    nc.sync.dma_start(out=idx_t[0:32, off:off + nseg * 8],
                                      in_=t_idx[:, off:off + nseg * 8])
                    _dma_gather_raw(
                        nc.gpsimd, g_t[:, :, :],
                        t_embs8[b * BANK: b * BANK + rows, 0:P],
                        idx_t[:, off:off + nseg * 8],
                        num_idxs=nseg * P, elem_size=P, elem_step=256,
                    )
                    g_tiles.append(g_t)
                if not gx_loaded:
                    # deferred so the first gathers go out first
                    nc.sync.dma_start(out=gx_t, in_=t_gx)
                    gx_loaded = True

                for u0 in range(0, nb, UNIT):
                    uw = min(UNIT, nb - u0)
                    psum_o = psb.tile([P, UNIT * P], mybir.dt.float32,
                                      space="PSUM")
                    for s in range(u0, u0 + uw):
                        cols = sched[g][s]
                        psum_u = psu.tile([P, P], mybir.dt.float32,
                                          space="PSUM")
                        nlast = len(cols) - 1
                        for j, (b, ch, cid) in enumerate(cols):
                            s_ap = get_s(cid)
                            nc.tensor.matmul(
                                out=psum_u[:, :],
                                lhsT=g_tiles[b][:, ch, :],
                                rhs=s_ap,
                                start=(j == 0), stop=(j == nlast),
                            )
                        u16 = xfer.tile([P, P], mybir.dt.float16, tag="u")
                        nc.scalar.activation(
                            out=u16, in_=psum_u,
                            func=mybir.ActivationFunctionType.Copy)
                        q = s - u0
                        nc.tensor.matmul(
                            out=psum_o[:, q * P:(q + 1) * P], lhsT=u16,
                            rhs=w_t, start=True, stop=True,
                            skip_group_check=True)
                    flush_pending()
                    state["pending"] = (psum_o, (st_g + u0) * P, uw * P, g)
            flush_pending()
            if state["wcol"] < npad:
                nc.sync.dma_start(out=t_out[:, state["wcol"]:],
                                  in_=out_t[:, state["wcol"]:])
    nc.compile()
    return nc


def _unshard(cfg, meta, res):
    TPC, NSB = cfg.TPC, cfg.NSB
    outs = []
    for c in range(cfg.NCORES):
        o = res.results[c]["out"]                       # [P, NSB*P] fp16
        o = np.asarray(o, np.float32).reshape(P, NSB, P)
        o = o.transpose(1, 0, 2).reshape(NSB * P, P)[:TPC]
        outs.append(o)
    virt = np.concatenate(outs, axis=0)
    return virt[meta["perm"]].astype(np.float32)


def kernel(X, embs, W, edge_index, edge_weight):
    cfg = _REAL
    meta, in_maps = _host_prep(cfg, X, embs, W, edge_index, edge_weight)
    nc = _build_program(cfg, meta)
    res = run_bass_kernel_spmd(nc, in_maps, list(range(cfg.NCORES)))
    return _unshard(cfg, meta, res)
